# revision 30
# baseline (speedup 1.0000x reference)
"""BertSelfAttention with gated prompt-prefix branch on 8 Trainium2 cores.

Sharding: data-parallel over batch (B=8 -> 1 batch element per core), no
collectives. Per core, the full attention pipeline runs in a transposed
[feature, seq] layout so that softmax statistics ride through the matmuls:

  qT/kT = W @ hsT          [768, 1024]  (bf16, PE)
  v_aug = hs @ WvT_aug     [1024, 780]  natural layout, 65-col stride per
                           head, col 65h+64 = ones (denominator column)
  scoresT_h = kh @ qh.T    [t, s] via K=64 row-tiled matmuls, 2 heads
                           concurrently on the 128x128 PE array
  expT = exp(SCALE*scoresT)  split between the ACT engine (true exp) and
         the DVE (Schraudolph bit-trick: bf16_bits = int16(x*a + b)) so
         that neither drain engine ever paces the PE.  The PE p-state
         ramp resets whenever a PE instruction blocks on an unsatisfied
         semaphore, halving matmul throughput; the whole schedule is
         built to keep every PE wait pre-satisfied.
  ctxT_aug_h = v_aug_h.T @ expT_h       rows 0..63 ctx, row 64 = sum_t exp
  ctx matmuls for pair c-1 interleave with scores for pair c (a full
  pair of lag), and the Q/K/V/prompt projections are spread through the
  pair-0 score phase as PE filler.
  out_h = ctxT/denom + pctxT/pdenom     (DVE mul from PSUM + GpSimd
          mul/add; reciprocal + partition-broadcast ride the DMA rings)

Output is produced as outT [768, 1024] fp32 per core; the host transposes
and stacks to [8, 1024, 768].
"""

import numpy as np
import ml_dtypes

import concourse.bass as bass
import concourse.mybir as mybir
import concourse.tile as tile
from concourse.bass_utils import run_bass_kernel_spmd
from concourse.vector_clock import ScopedClock


class SplitDrainTileContext(tile.TileContext):
    """This walrus build rejects >2 sync waits on the kernel-tail Drain
    ("Too many sync wait commands"); split them across SP nops instead."""

    def _drain_and_barrier(self, tick_clock, wait_clock):
        probe = self.nc.sync.nop(nofuse=True, hint="drain_wait_split")
        wait_clock.add_sem_waits(
            probe.ins, ScopedClock({None: tick_clock.global_clock})
        )
        waits = list(probe.ins.sync_info.on_wait or [])
        if len(waits) > 1:
            probe.ins.sync_info.on_wait = waits[:1]
            for i in range(1, len(waits)):
                extra = self.nc.sync.nop(nofuse=True, hint="drain_wait_split")
                extra.ins.sync_info = mybir.SyncInfo(
                    on_wait=waits[i : i + 1], on_update=[]
                )
        drain_inst = self.nc.sync.drain()
        if drain_inst.ins.sync_info is not None:
            drain_inst.ins.sync_info.on_wait = []
        self.nc.all_engine_barrier()
        assert self.sems is not None
        popped = self.nc._tile_sem_poison_stack.pop()
        assert popped is self._sem_poison
        self.nc.clear_and_free_semaphores(list(self.sems.allocated().values()))
        self.nc.all_engine_barrier()

F32 = mybir.dt.float32
BF16 = mybir.dt.bfloat16
I16 = mybir.dt.int16
AF = mybir.ActivationFunctionType
ALU = mybir.AluOpType

H, DH, D = 12, 64, 768
S, AT, B = 1024, 64, 8
SCALE = 1.0 / np.sqrt(DH)
NC_D = D // 128  # 6 chunks over feature dim
NC_S = S // 128  # 8 chunks over sequence dim
PAIRS = H // 2  # 6 head pairs
VW = H * (DH + 1)  # 780: v with per-head ones column

# Schraudolph exp -> bf16 bits: bits = trunc(x * EXP_A + EXP_B), bitcast
# int16 -> bf16 gives ~exp(SCALE*x) with ~1.8% rms error that averages
# out inside the softmax-weighted context sum.
EXP_A = float(SCALE * 128.0 / np.log(2.0))
EXP_B = 16256.0 - 6.75

_CACHE = {}
LAST_RESULTS = None


def _exp_engine(tci, half):
    # DVE (Schraudolph) takes the half-1 exp tiles; ACT the half-0 tiles
    # plus the prefix exp and the denominator-row evacuations.
    return ("dve" if half == 1 else "act", False)


def _split_sync_waits(nc, cap=1):
    """Walrus on this image allows very few sync-wait commands per
    instruction (tensor_scalar rejects 2). Hoist excess waits onto
    same-engine nops placed immediately before the instruction."""
    for bb in nc.main_func.blocks:
        cur = list(bb.instructions)
        out = []
        for inst in cur:
            si = inst.sync_info
            waits = list(si.on_wait) if si and si.on_wait else []
            if len(waits) > cap:
                for i in range(0, len(waits) - cap):
                    bi = nc.engines[inst.engine].nop(
                        nofuse=True, hint="wait_split")
                    popped = nc.cur_bb.bb.instructions.pop()
                    assert popped is bi.ins
                    bi.ins.sync_info = mybir.SyncInfo(
                        on_wait=waits[i : i + 1], on_update=[])
                    out.append(bi.ins)
                si.on_wait = waits[len(waits) - cap:]
            out.append(inst)
        bb.instructions[:] = out


def _build_nc():
    nc = bass.Bass()
    hsT = nc.dram_tensor("hsT", [D, S], BF16, kind="ExternalInput")
    wqT = nc.dram_tensor("wqT", [D, D], BF16, kind="ExternalInput")
    wkT = nc.dram_tensor("wkT", [D, D], BF16, kind="ExternalInput")
    wvT = nc.dram_tensor("wvT", [D, VW], BF16, kind="ExternalInput")
    bq = nc.dram_tensor("bq", [D, 1], F32, kind="ExternalInput")
    bk = nc.dram_tensor("bk", [D, 1], F32, kind="ExternalInput")
    bvaug = nc.dram_tensor("bvaug", [128, VW], F32, kind="ExternalInput")
    promptT = nc.dram_tensor("promptT", [D, AT], BF16, kind="ExternalInput")
    mask = nc.dram_tensor("mask", [S, 1], F32, kind="ExternalInput")
    gating = nc.dram_tensor("gating", [128, VW], F32, kind="ExternalInput")
    outT = nc.dram_tensor("outT", [D, S], F32, kind="ExternalOutput")

    with SplitDrainTileContext(nc) as tc:
        _emit(nc, tc, hsT, wqT, wkT, wvT, bq, bk, bvaug, promptT, mask,
              gating, outT)
    _split_sync_waits(nc)
    return nc


def _emit(nc, tc, hsT, wqT, wkT, wvT, bq, bk, bvaug, promptT, mask, gating,
          outT):
    from contextlib import ExitStack

    with ExitStack() as ctx:
        pers = ctx.enter_context(tc.tile_pool(name="pers", bufs=1))

        # ---- SBUF arrays that live into the attention phase ----
        mask_sb = pers.tile([128, NC_S], F32, tag="mask")
        emask_sb = pers.tile([128, NC_S], F32, tag="emask")
        qT_sb = pers.tile([128, NC_D * S], BF16, tag="qT")
        kT_sb = pers.tile([128, NC_D * S], BF16, tag="kT")
        v_sb = pers.tile([128, NC_S * VW], BF16, tag="v")
        pkT_sb = pers.tile([128, NC_D * AT], BF16, tag="pkT")
        pv_sb = pers.tile([128, VW], BF16, tag="pv")

        # ---- projection-phase-only arrays (pool closed afterwards so the
        # attention pools can reuse the space) ----
        proj_cm = tc.tile_pool(name="proj", bufs=1, side="right")
        proj = proj_cm.__enter__()
        hsT_sb = proj.tile([128, NC_D * S], BF16, tag="hsT")
        wqT_sb = proj.tile([128, NC_D * D], BF16, tag="wqT")
        wkT_sb = proj.tile([128, NC_D * D], BF16, tag="wkT")
        wvT_sb = proj.tile([128, NC_D * VW], BF16, tag="wvT")
        pT_sb = proj.tile([128, NC_D * AT], BF16, tag="pT")
        bq_sb = proj.tile([128, NC_D], F32, tag="bq")
        bk_sb = proj.tile([128, NC_D], F32, tag="bk")
        bvaug_sb = proj.tile([128, VW], F32, tag="bvaug")
        graw_sb = proj.tile([128, VW], F32, tag="graw")
        gbc_sb = proj.tile([128, VW], F32, tag="gbc")
        pvtmp_sb = proj.tile([64, VW], F32, tag="pvtmp")

        for src, dst, w in ((wqT, wqT_sb, D), (hsT, hsT_sb, S),
                            (wkT, wkT_sb, D), (wvT, wvT_sb, VW),
                            (promptT, pT_sb, AT)):
            nc.sync.dma_start(
                dst[:].rearrange("p (c s) -> p c s", s=w),
                src[:, :].rearrange("(c p) s -> p c s", p=128))
        # biases / mask: [768,1] & [1024,1] -> [128, nchunks]
        nc.sync.dma_start(bq_sb[:], bq.rearrange("(c p) 1 -> p c", p=128))
        nc.sync.dma_start(bk_sb[:], bk.rearrange("(c p) 1 -> p c", p=128))
        nc.sync.dma_start(mask_sb[:], mask.rearrange("(c p) 1 -> p c", p=128))
        nc.sync.dma_start(bvaug_sb[:], bvaug[:])
        # gating arrives host-replicated to [128, 780] (65 copies per head
        # along the row, broadcast down the partitions)
        nc.sync.dma_start(graw_sb[:], gating[:])
        # tanh, then force the ones-column slots back to 1.0
        nc.scalar.activation(gbc_sb[:], graw_sb[:], AF.Tanh)
        ones_slots = gbc_sb[:, :].rearrange(
            "p (h e) -> p h e", h=H)[:, :, DH:DH + 1]
        nc.vector.memset(ones_slots, 1.0)
        # e^mask, folded into the V rows (incl. ones column) instead of an
        # exp bias: exp(S*x + m_t) == e^{m_t} * exp(S*x), and the ones
        # column then accumulates the correctly-masked denominator.
        nc.scalar.activation(emask_sb[:], mask_sb[:], AF.Exp)

        # SBUF pools that outlive the projection phase — opened before the
        # closeable PSUM pools so the per-side pool stack unwinds LIFO
        exp_pool = ctx.enter_context(tc.tile_pool(name="expp", bufs=4))
        pexp_pool = ctx.enter_context(tc.tile_pool(name="pexpp", bufs=3))

        # ---- PSUM pools for the projection phase (closed afterwards) ----
        mm_cm = tc.tile_pool(name="mm", bufs=2, space="PSUM")
        mm_pool = mm_cm.__enter__()
        sc0_cm = tc.tile_pool(name="scp0", bufs=2, space="PSUM")
        scp = {"p": sc0_cm.__enter__()}

        def emit_exp(dst, src, engine):
            if engine == "act":
                nc.scalar.activation(dst, src, AF.Exp, scale=SCALE)
            else:
                nc.vector.tensor_scalar(
                    dst.bitcast(I16), src, EXP_A, EXP_B,
                    op0=ALU.mult, op1=ALU.add)

        def scores_tci(c, tci, exp_ab):
            """Scores + exp for (pair c, key-chunk tci), 2 heads row-tiled."""
            for half in range(2):
                hp = half * 64
                st = scp["p"].tile([128, S], F32, tag="sc",
                                   name=f"st_{c}_{tci}_{half}")
                lhsT = kT_sb[hp:hp + 64,
                             c * S + tci * 128: c * S + (tci + 1) * 128]
                eng, split = _exp_engine(tci, half)
                dst = exp_ab[half][:, tci * S:(tci + 1) * S]
                for sb in range(2):
                    nc.tensor.matmul(
                        st[:, sb * 512:(sb + 1) * 512], lhsT,
                        qT_sb[hp:hp + 64,
                              c * S + sb * 512: c * S + (sb + 1) * 512],
                        tile_position=(hp, 0))
                    if split:
                        emit_exp(dst[:, sb * 512:(sb + 1) * 512],
                                 st[:, sb * 512:(sb + 1) * 512], eng)
                if not split:
                    emit_exp(dst, st[:], eng)

        # Prefix psum tiles come from a separate region (never the score
        # rotation: an odd alloc inserted there de-phases the 2-slot
        # lookahead and every following score matmul blocks on a
        # 1-slot-old drain). In the projection phase that region is the
        # sc0 pool; in the attention phase it is the ctx-h1 bank pair,
        # which is idle during the first half of each block.
        pfx = {"pool": lambda name: scp["p"].tile([128, S], F32, tag="sc",
                                                  name=name)}

        def prefix_scores(c, pexp):
            """Both heads' prefix scores (4 matmuls) + one exp."""
            psp = pfx["pool"](f"psp_{c}")
            for half in range(2):
                hp = half * 64
                for sb in range(2):
                    nc.tensor.matmul(
                        psp[hp:hp + 64, sb * 512:(sb + 1) * 512],
                        pkT_sb[hp:hp + 64, c * AT:(c + 1) * AT],
                        qT_sb[hp:hp + 64,
                              c * S + sb * 512: c * S + (sb + 1) * 512],
                        tile_position=(hp, hp))
            nc.scalar.activation(pexp[:], psp[:], AF.Exp, scale=SCALE)

        # ---- projection helpers (run as PE filler between score chunks) ----
        def qk_chain(c, w_sb, b_sb, o_sb):
            ps = mm_pool.tile([128, S], F32, tag="mm")
            for kc in range(NC_D):
                lhsT = w_sb[:, kc * D + c * 128: kc * D + (c + 1) * 128]
                for sb in range(2):
                    nc.tensor.matmul(
                        ps[:, sb * 512:(sb + 1) * 512], lhsT,
                        hsT_sb[:, kc * S + sb * 512: kc * S + (sb + 1) * 512],
                        start=(kc == 0), stop=(kc == NC_D - 1))
            nc.vector.tensor_scalar_add(o_sb[:, c * S:(c + 1) * S],
                                        ps[:], b_sb[:, c:c + 1])

        def v_chunk(sc):
            ps = mm_pool.tile([128, S], F32, tag="mm")
            for kc in range(NC_D):
                lhsT = hsT_sb[:, kc * S + sc * 128: kc * S + (sc + 1) * 128]
                nc.tensor.matmul(ps[:, 0:512], lhsT,
                                 wvT_sb[:, kc * VW: kc * VW + 512],
                                 start=(kc == 0), stop=(kc == NC_D - 1))
                nc.tensor.matmul(ps[:, 512:VW], lhsT,
                                 wvT_sb[:, kc * VW + 512: (kc + 1) * VW],
                                 start=(kc == 0), stop=(kc == NC_D - 1))
            vt = proj.tile([128, VW], F32, tag="vtmp", name=f"vt{sc}",
                           bufs=2)
            nc.vector.tensor_add(vt[:], ps[:, 0:VW], bvaug_sb[:])
            nc.vector.tensor_scalar_mul(v_sb[:, sc * VW:(sc + 1) * VW],
                                        vt[:], emask_sb[:, sc:sc + 1])

        # ---- projection phase: pair-0 scores ride between filler chains ----
        qk_chain(0, wqT_sb, bq_sb, qT_sb)
        qk_chain(0, wkT_sb, bk_sb, kT_sb)
        fillers = []
        for c in range(1, NC_D):
            fillers.append(lambda c=c: qk_chain(c, wqT_sb, bq_sb, qT_sb))
            fillers.append(lambda c=c: qk_chain(c, wkT_sb, bk_sb, kT_sb))
        exps = {0: [exp_pool.tile([128, NC_S * S], BF16, tag="exp",
                                  name=f"exp_0_{i}") for i in range(2)]}
        fi = 0
        for tci in range(NC_S):
            scores_tci(0, tci, exps[0])
            n = 2 if tci < 2 else 1
            for _ in range(n):
                if fi < len(fillers):
                    fillers[fi]()
                    fi += 1
            v_chunk(tci)
        while fi < len(fillers):
            fillers[fi]()
            fi += 1

        # ---- prompt K projection (transposed) ----
        for c in range(NC_D):
            ps = mm_pool.tile([128, S], F32, tag="mm")
            for kc in range(NC_D):
                nc.tensor.matmul(
                    ps[:, 0:AT],
                    wkT_sb[:, kc * D + c * 128: kc * D + (c + 1) * 128],
                    pT_sb[:, kc * AT:(kc + 1) * AT],
                    start=(kc == 0), stop=(kc == NC_D - 1))
            nc.vector.tensor_scalar_add(pkT_sb[:, c * AT:(c + 1) * AT],
                                        ps[:, 0:AT], bk_sb[:, c:c + 1])

        # pair-0 prefix scores before the prompt-V chain so pexp(0) lands
        # well before the first attention block consumes the banks
        pexps = {0: pexp_pool.tile([128, S], BF16, tag="pexp", name="pexp0")}
        prefix_scores(0, pexps[0])

        # ---- prompt V projection (natural, gate-scaled, duplicated) ----
        ps = mm_pool.tile([128, S], F32, tag="mm")
        for kc in range(NC_D):
            lhsT = pT_sb[:, kc * AT:(kc + 1) * AT]
            nc.tensor.matmul(ps[0:AT, 0:512], lhsT,
                             wvT_sb[:, kc * VW: kc * VW + 512],
                             start=(kc == 0), stop=(kc == NC_D - 1))
            nc.tensor.matmul(ps[0:AT, 512:VW], lhsT,
                             wvT_sb[:, kc * VW + 512: (kc + 1) * VW],
                             start=(kc == 0), stop=(kc == NC_D - 1))
        nc.vector.tensor_add(pvtmp_sb[:], ps[0:AT, 0:VW], bvaug_sb[0:AT, :])
        nc.vector.tensor_mul(pv_sb[0:AT, :], pvtmp_sb[:], gbc_sb[0:AT, :])
        nc.sync.dma_start(pv_sb[AT:128, :], pv_sb[0:AT, :])

        sc0_cm.__exit__(None, None, None)
        proj_cm.__exit__(None, None, None)
        mm_cm.__exit__(None, None, None)

        # ---- attention-phase pools (reuse the projection PSUM) ----
        scp["p"] = ctx.enter_context(
            tc.tile_pool(name="scp", bufs=2, space="PSUM"))
        ctx_pool = ctx.enter_context(
            tc.tile_pool(name="ctxp", bufs=1, space="PSUM"))
        pfx["pool"] = lambda name: ctx_pool.tile([128, S], F32, tag="ctx1",
                                                 name=name)
        norm_pool = ctx.enter_context(tc.tile_pool(name="normp", bufs=2))
        out_pool = ctx.enter_context(tc.tile_pool(name="outp", bufs=2))
        dscr_pool = ctx.enter_context(
            tc.tile_pool(name="dscr", bufs=2, space="DRAM"))

        cps = {}       # pair -> [2 ctx psum accumulators]
        evstate = {}   # pair -> list of per-half finish state

        def ctx_mm(c, half, tci):
            h = 2 * c + half
            lhsT = v_sb[:, tci * VW + h * 65: tci * VW + h * 65 + 65]
            for sb in range(2):
                nc.tensor.matmul(
                    cps[c][half][0:65, sb * 512:(sb + 1) * 512], lhsT,
                    exps[c][half][:, tci * S + sb * 512:
                                  tci * S + (sb + 1) * 512],
                    start=(tci == 0), stop=(tci == NC_S - 1))

        def stage1_prefix(c, half):
            """Prefix-ctx matmuls + fast psum evacuation on ACT (the
            prefix denominator rides in row 64 of the copy). Owns the
            evstate entry."""
            h = 2 * c + half
            hp = half * 64
            pps = pfx["pool"](f"pps_{c}_{half}")
            for sb in range(2):
                nc.tensor.matmul(
                    pps[0:65, sb * 512:(sb + 1) * 512],
                    pv_sb[hp:hp + 64, h * 65: h * 65 + 65],
                    pexps[c][hp:hp + 64, sb * 512:(sb + 1) * 512],
                    tile_position=(hp, 0))
            pe_ev = norm_pool.tile([65, S], F32, tag="pe_ev", bufs=4,
                                   name=f"pe_{c}_{half}")
            nc.scalar.copy(pe_ev[:], pps[0:65, :])
            dresh = norm_pool.tile([128, 16], F32, tag="dresh", bufs=4,
                                   name=f"dr_{c}_{half}")
            nc.sync.dma_start(dresh[:, 8:16], pe_ev[64:65, :])
            if half == 0:
                evstate[c] = []
            evstate[c].append(
                {"dresh": dresh, "cps": cps[c][half], "pe_ev": pe_ev})

        def stage1_dens(c, half):
            """Main-ctx denominator row -> SBUF (ACT; DMA cannot read
            PSUM) -> DMA-reshape across partitions. Runs right after
            ctx(c, half)'s last accumulation matmul."""
            st = evstate[c][half]
            den_c = norm_pool.tile([1, S], F32, tag="den", bufs=4,
                                   name=f"den_{c}_{half}")
            nc.scalar.copy(den_c[:], cps[c][half][64:65, :])
            nc.sync.dma_start(st["dresh"][:, 0:8], den_c[:])

        def stage1_recip(c, half):
            """Reciprocal of one head's denominators, broadcast via DRAM."""
            st = evstate[c][half]
            rrec = norm_pool.tile([128, 16], F32, tag="rrec", bufs=4,
                                  name=f"rr_{c}_{half}")
            nc.vector.reciprocal(rrec[:], st["dresh"][:])
            r_d = dscr_pool.tile([1, 2 * S], F32, tag="rd", bufs=4,
                                 name=f"rd_{c}_{half}")
            nc.sync.dma_start(r_d[0:1, 0:S], rrec[:, 0:8])
            nc.sync.dma_start(r_d[0:1, S:2 * S], rrec[:, 8:16])
            r_bc = norm_pool.tile([64, 2 * S], F32, tag="rbc", bufs=4,
                                  name=f"rbc_{c}_{half}")
            r_src = bass.AP(r_d[:].tensor, r_d[:].offset,
                            [[0, 64], [1, 2 * S]])
            nc.sync.dma_start(r_bc[:], r_src)
            st["r_bc"] = r_bc

        def stage1_cemul(c, half):
            """Normalize main ctx straight out of PSUM (frees the ctx
            accumulator for the next pair)."""
            h = 2 * c + half
            st = evstate[c][half]
            ce_n = norm_pool.tile([64, S], F32, tag="ce", bufs=4,
                                  name=f"ce_{h}")
            nc.vector.tensor_mul(ce_n[:], st["cps"][0:64, :],
                                 st["r_bc"][:, 0:S])
            st["ce_n"] = ce_n

        def stage2_thunks(c):
            """Prefix normalize + combine + store for pair c, spread
            through the following block on GpSimd."""
            thunks = []
            for half in range(2):
                h = 2 * c + half
                st = evstate[c][half]
                def t(h=h, st=st):
                    pe_n = out_pool.tile([64, S], F32, tag="pe", bufs=2,
                                         name=f"pen_{h}")
                    nc.gpsimd.tensor_mul(pe_n[:], st["pe_ev"][0:64, :],
                                         st["r_bc"][:, S:2 * S])
                    ot = out_pool.tile([64, S], F32, tag="ot", bufs=2,
                                       name=f"ot_{h}")
                    nc.gpsimd.tensor_add(ot[:], st["ce_n"][:], pe_n[:])
                    nc.sync.dma_start(outT[h * 64:(h + 1) * 64, :], ot[:])
                thunks.append(t)
            return thunks

        def attention_block(c):
            """Scores for pair c interleaved with the finish of pair c-1.
            ctx(c-1) h0 runs over tci 0-3 and h1 over tci 4-7, so each
            half's denominator chain lands while the other half keeps the
            PE cadence uniform (~8-10 matmuls per tci, which keeps every
            score-psum rotation wait pre-satisfied). Each half's ctx psum
            bank is normalized (and thus freed) one half-block before the
            next pair's accumulation reaches it."""
            p = c - 1
            cps[p] = [
                ctx_pool.tile([65, S], F32, tag="ctx0", name=f"cps_{p}_0"),
                None,  # h1 allocated at tci 5, after the prefix tiles
            ]
            s2 = stage2_thunks(c - 2) if c >= 2 else []
            for tci in range(NC_S):
                scores_tci(c, tci, exps[c])
                if tci < 4:
                    ctx_mm(p, 0, 2 * tci)
                    ctx_mm(p, 0, 2 * tci + 1)
                if tci == 0:
                    if c >= 2:
                        stage1_recip(c - 2, 1)
                elif tci == 1:
                    if c >= 2:
                        stage1_cemul(c - 2, 1)
                    if s2:
                        s2[0]()
                elif tci == 2:
                    stage1_prefix(p, 0)
                elif tci == 3:
                    stage1_prefix(p, 1)
                    stage1_dens(p, 0)
                elif tci == 4:
                    prefix_scores(c, pexps[c])
                    stage1_recip(p, 0)
                elif tci == 5:
                    cps[p][1] = ctx_pool.tile([128, S], F32, tag="ctx1",
                                              name=f"cps_{p}_1")
                    evstate[p][1]["cps"] = cps[p][1]
                    ctx_mm(p, 1, 0)
                    ctx_mm(p, 1, 1)
                    ctx_mm(p, 1, 2)
                    stage1_cemul(p, 0)
                    if s2:
                        s2[1]()
                elif tci == 6:
                    ctx_mm(p, 1, 3)
                    ctx_mm(p, 1, 4)
                elif tci == 7:
                    ctx_mm(p, 1, 5)
                    ctx_mm(p, 1, 6)
                    ctx_mm(p, 1, 7)
                    stage1_dens(p, 1)

        for c in range(1, PAIRS):
            exps[c] = [exp_pool.tile([128, NC_S * S], BF16, tag="exp",
                                     name=f"exp_{c}_{i}") for i in range(2)]
            pexps[c] = pexp_pool.tile([128, S], BF16, tag="pexp",
                                      name=f"pexp_{c}")
            attention_block(c)

        # ---- trailing block: ctx(5) dense + pending finishes ----
        c = PAIRS - 1
        cps[c] = [ctx_pool.tile([65, S], F32, tag="ctx0",
                                name=f"cps_{c}_0"), None]
        s2 = stage2_thunks(c - 1)
        for slot in range(NC_S):
            if slot < 4:
                ctx_mm(c, 0, 2 * slot)
                ctx_mm(c, 0, 2 * slot + 1)
            if slot == 0:
                stage1_recip(c - 1, 1)
            elif slot == 1:
                stage1_cemul(c - 1, 1)
                s2[0]()
            elif slot == 2:
                stage1_prefix(c, 0)
            elif slot == 3:
                stage1_prefix(c, 1)
                stage1_dens(c, 0)
            elif slot == 4:
                stage1_recip(c, 0)
                cps[c][1] = ctx_pool.tile([128, S], F32, tag="ctx1",
                                          name=f"cps_{c}_1")
                evstate[c][1]["cps"] = cps[c][1]
                for k in range(4):
                    ctx_mm(c, 1, k)
            elif slot == 5:
                stage1_cemul(c, 0)
                s2[1]()
                for k in range(4, 8):
                    ctx_mm(c, 1, k)
                stage1_dens(c, 1)
        stage1_recip(c, 1)
        stage1_cemul(c, 1)
        for t in stage2_thunks(c):
            t()


def _prep_inputs(hidden_states, prompt_tokens, gating_factor, attention_mask,
                 Wq, bq, Wk, bk, Wv, bv):
    bf = ml_dtypes.bfloat16
    hs = np.asarray(hidden_states, np.float32)
    mask = np.asarray(attention_mask, np.float32).reshape(B, S)
    wqT = np.ascontiguousarray(np.asarray(Wq, np.float32).T).astype(bf)
    wkT = np.ascontiguousarray(np.asarray(Wk, np.float32).T).astype(bf)
    # augmented WvT: [din, 780], col 65h+j = Wv.T[:, 64h+j], col 65h+64 = 0
    wvT_f = np.asarray(Wv, np.float32).T  # [din, dout]
    wvT_aug = np.zeros((D, VW), np.float32)
    idx = np.arange(D)
    aug_cols = (idx // DH) * (DH + 1) + (idx % DH)
    wvT_aug[:, aug_cols] = wvT_f
    wvT_aug = wvT_aug.astype(bf)
    bq_c = np.asarray(bq, np.float32).reshape(D, 1)
    bk_c = np.asarray(bk, np.float32).reshape(D, 1)
    bv_aug = np.zeros(VW, np.float32)
    bv_aug[aug_cols] = np.asarray(bv, np.float32)
    bv_aug[DH::DH + 1] = 1.0
    bvaug_bc = np.ascontiguousarray(
        np.broadcast_to(bv_aug, (128, VW)), np.float32)
    pT = np.ascontiguousarray(
        np.asarray(prompt_tokens, np.float32)[0].T).astype(bf)
    gat_row = np.repeat(
        np.asarray(gating_factor, np.float32).reshape(H), DH + 1)
    gat = np.ascontiguousarray(
        np.broadcast_to(gat_row, (128, VW)), np.float32)

    shared = dict(wqT=wqT, wkT=wkT, wvT=wvT_aug, bq=bq_c, bk=bk_c,
                  bvaug=bvaug_bc, promptT=pT, gating=gat)
    in_maps = []
    for b in range(B):
        m = dict(shared)
        m["hsT"] = np.ascontiguousarray(hs[b].T).astype(bf)
        m["mask"] = np.ascontiguousarray(mask[b].reshape(S, 1))
        in_maps.append(m)
    return in_maps


def kernel(**inputs):
    global LAST_RESULTS
    if "nc" not in _CACHE:
        _CACHE["nc"] = _build_nc()
    nc = _CACHE["nc"]
    in_maps = _prep_inputs(**inputs)
    res = None
    for attempt in range(3):
        try:
            res = run_bass_kernel_spmd(nc, in_maps, list(range(B)))
            break
        except ModuleNotFoundError:
            # BASS_TRACE set but this image lacks antenv.axon_hooks
            import os

            os.environ["BASS_NEVER_TRACE"] = "1"
            if attempt == 2:
                raise
        except Exception:
            # transient NRT_EXEC_UNIT_UNRECOVERABLE on a cold device has
            # been observed; a retry on the same session recovers
            if attempt == 2:
                raise
    LAST_RESULTS = res
    out = np.empty((B, S, D), np.float32)
    for b in range(B):
        out[b] = res.results[b]["outT"].T
    return out


# revision 35
# speedup vs baseline: 1.0185x; 1.0185x over previous
"""BertSelfAttention with gated prompt-prefix branch on 8 Trainium2 cores.

Sharding: data-parallel over batch (B=8 -> 1 batch element per core), no
collectives. Per core, the full attention pipeline runs in a transposed
[feature, seq] layout so that softmax statistics ride through the matmuls:

  qT/kT = W @ hsT          [768, 1024]  (bf16, PE)
  v_aug = hs @ WvT_aug     [1024, 780]  natural layout, 65-col stride per
                           head, col 65h+64 = ones (denominator column)
  scoresT_h = kh @ qh.T    [t, s] via K=64 row-tiled matmuls, 2 heads
                           concurrently on the 128x128 PE array
  expT = exp(SCALE*scoresT + mask[t])   one fused ACT op per tile
  ctxT_aug_h = v_aug_h.T @ expT_h       rows 0..63 ctx, row 64 = sum_t exp
  prefix branch identical with prompt-derived k/v; tanh(gate) folded into
  the prefix v weights on-device
  out_h = ctxT/denom + pctxT/pdenom     (DVE, reciprocal + partition bcast)

Output is produced as outT [768, 1024] fp32 per core; the host transposes
and stacks to [8, 1024, 768].
"""

import numpy as np
import ml_dtypes

import concourse.bass as bass
import concourse.mybir as mybir
import concourse.tile as tile
from concourse.bass_utils import run_bass_kernel_spmd
from concourse.vector_clock import ScopedClock


class SplitDrainTileContext(tile.TileContext):
    """This walrus build rejects >2 sync waits on the kernel-tail Drain
    ("Too many sync wait commands"); split them across SP nops instead."""

    def _drain_and_barrier(self, tick_clock, wait_clock):
        probe = self.nc.sync.nop(nofuse=True, hint="drain_wait_split")
        wait_clock.add_sem_waits(
            probe.ins, ScopedClock({None: tick_clock.global_clock})
        )
        waits = list(probe.ins.sync_info.on_wait or [])
        if len(waits) > 1:
            probe.ins.sync_info.on_wait = waits[:1]
            for i in range(1, len(waits)):
                extra = self.nc.sync.nop(nofuse=True, hint="drain_wait_split")
                extra.ins.sync_info = mybir.SyncInfo(
                    on_wait=waits[i : i + 1], on_update=[]
                )
        drain_inst = self.nc.sync.drain()
        if drain_inst.ins.sync_info is not None:
            drain_inst.ins.sync_info.on_wait = []
        self.nc.all_engine_barrier()
        assert self.sems is not None
        popped = self.nc._tile_sem_poison_stack.pop()
        assert popped is self._sem_poison
        self.nc.clear_and_free_semaphores(list(self.sems.allocated().values()))
        self.nc.all_engine_barrier()

F32 = mybir.dt.float32
BF16 = mybir.dt.bfloat16
I16 = mybir.dt.int16
AF = mybir.ActivationFunctionType
ALU = mybir.AluOpType

H, DH, D = 12, 64, 768
S, AT, B = 1024, 64, 8
SCALE = 1.0 / np.sqrt(DH)
NC_D = D // 128  # 6 chunks over feature dim
NC_S = S // 128  # 8 chunks over sequence dim
PAIRS = H // 2  # 6 head pairs
VW = H * (DH + 1)  # 780: v with per-head ones column

# Schraudolph exp -> bf16 bits: bits = trunc(x * EXP_A + EXP_B); bitcasting
# int16 -> bf16 gives ~exp(SCALE*x) with ~1.8% rms error that washes out in
# the softmax-weighted context sum. Half the exp tiles run this way on the
# DVE so the ACT engine is never the per-iteration pacer: a PE instruction
# whose semaphore wait is unsatisfied at decode resets the tensor engine's
# p-state ramp (3us of continuous execution to reach 2.4GHz, else 1.2GHz),
# so every drain must stay ahead of the score-psum rotation.
EXP_A = float(SCALE * 128.0 / np.log(2.0))
EXP_B = 16256.0 - 6.75

_CACHE = {}
LAST_RESULTS = None


def _split_sync_waits(nc, cap=1):
    """Walrus on this image allows very few sync-wait commands per
    instruction (tensor_scalar rejects 2). Hoist excess waits onto
    same-engine nops placed immediately before the instruction."""
    for bb in nc.main_func.blocks:
        cur = list(bb.instructions)
        out = []
        for inst in cur:
            si = inst.sync_info
            waits = list(si.on_wait) if si and si.on_wait else []
            if len(waits) > cap:
                for i in range(0, len(waits) - cap):
                    bi = nc.engines[inst.engine].nop(
                        nofuse=True, hint="wait_split")
                    popped = nc.cur_bb.bb.instructions.pop()
                    assert popped is bi.ins
                    bi.ins.sync_info = mybir.SyncInfo(
                        on_wait=waits[i : i + 1], on_update=[])
                    out.append(bi.ins)
                si.on_wait = waits[len(waits) - cap:]
            out.append(inst)
        bb.instructions[:] = out


def _build_nc():
    nc = bass.Bass()
    hsT = nc.dram_tensor("hsT", [D, S], BF16, kind="ExternalInput")
    wqT = nc.dram_tensor("wqT", [D, D], BF16, kind="ExternalInput")
    wkT = nc.dram_tensor("wkT", [D, D], BF16, kind="ExternalInput")
    wvT = nc.dram_tensor("wvT", [D, VW], BF16, kind="ExternalInput")
    bq = nc.dram_tensor("bq", [D, 1], F32, kind="ExternalInput")
    bk = nc.dram_tensor("bk", [D, 1], F32, kind="ExternalInput")
    bvaug = nc.dram_tensor("bvaug", [128, VW], F32, kind="ExternalInput")
    promptT = nc.dram_tensor("promptT", [D, AT], BF16, kind="ExternalInput")
    mask = nc.dram_tensor("mask", [S, 1], F32, kind="ExternalInput")
    gating = nc.dram_tensor("gating", [128, VW], F32, kind="ExternalInput")
    outT = nc.dram_tensor("outT", [D, S], F32, kind="ExternalOutput")

    with SplitDrainTileContext(nc) as tc:
        _emit(nc, tc, hsT, wqT, wkT, wvT, bq, bk, bvaug, promptT, mask,
              gating, outT)
    _split_sync_waits(nc)
    return nc


def _emit(nc, tc, hsT, wqT, wkT, wvT, bq, bk, bvaug, promptT, mask, gating,
          outT):
    from contextlib import ExitStack

    with ExitStack() as ctx:
        pers = ctx.enter_context(tc.tile_pool(name="pers", bufs=1))

        # ---- SBUF arrays that live into the attention phase ----
        mask_sb = pers.tile([128, NC_S], F32, tag="mask")
        emask_sb = pers.tile([128, NC_S], F32, tag="emask")
        qT_sb = pers.tile([128, NC_D * S], BF16, tag="qT")
        kT_sb = pers.tile([128, NC_D * S], BF16, tag="kT")
        v_sb = pers.tile([128, NC_S * VW], BF16, tag="v")
        pkT_sb = pers.tile([128, NC_D * AT], BF16, tag="pkT")
        pv_sb = pers.tile([128, VW], BF16, tag="pv")

        # ---- projection-phase-only arrays (pool closed afterwards so the
        # attention pools can reuse the space) ----
        proj_cm = tc.tile_pool(name="proj", bufs=1, side="right")
        proj = proj_cm.__enter__()
        hsT_sb = proj.tile([128, NC_D * S], BF16, tag="hsT")
        wqT_sb = proj.tile([128, NC_D * D], BF16, tag="wqT")
        wkT_sb = proj.tile([128, NC_D * D], BF16, tag="wkT")
        wvT_sb = proj.tile([128, NC_D * VW], BF16, tag="wvT")
        pT_sb = proj.tile([128, NC_D * AT], BF16, tag="pT")
        bq_sb = proj.tile([128, NC_D], F32, tag="bq")
        bk_sb = proj.tile([128, NC_D], F32, tag="bk")
        bvaug_sb = proj.tile([128, VW], F32, tag="bvaug")
        graw_sb = proj.tile([128, VW], F32, tag="graw")
        gbc_sb = proj.tile([128, VW], F32, tag="gbc")
        pvtmp_sb = proj.tile([64, VW], F32, tag="pvtmp")

        for src, dst, w in ((wqT, wqT_sb, D), (wkT, wkT_sb, D),
                            (hsT, hsT_sb, S), (wvT, wvT_sb, VW),
                            (promptT, pT_sb, AT)):
            nc.sync.dma_start(
                dst[:].rearrange("p (c s) -> p c s", s=w),
                src[:, :].rearrange("(c p) s -> p c s", p=128))
        # biases / mask: [768,1] & [1024,1] -> [128, nchunks]
        nc.sync.dma_start(bq_sb[:], bq.rearrange("(c p) 1 -> p c", p=128))
        nc.sync.dma_start(bk_sb[:], bk.rearrange("(c p) 1 -> p c", p=128))
        nc.sync.dma_start(mask_sb[:], mask.rearrange("(c p) 1 -> p c", p=128))
        nc.sync.dma_start(bvaug_sb[:], bvaug[:])
        # gating arrives host-replicated to [128, 780] (65 copies per head
        # along the row, broadcast down the partitions)
        nc.sync.dma_start(graw_sb[:], gating[:])
        # tanh, then force the ones-column slots back to 1.0
        nc.scalar.activation(gbc_sb[:], graw_sb[:], AF.Tanh)
        ones_slots = gbc_sb[:, :].rearrange(
            "p (h e) -> p h e", h=H)[:, :, DH:DH + 1]
        nc.vector.memset(ones_slots, 1.0)
        # e^mask, folded into the V rows (incl. ones column) instead of an
        # exp bias: exp(S*x + m_t) == e^{m_t} * exp(S*x), and the ones
        # column then accumulates the correctly-masked denominator.
        nc.scalar.activation(emask_sb[:], mask_sb[:], AF.Exp)

        # SBUF pools that outlive the projection phase — opened before the
        # closeable PSUM pools so the per-side pool stack unwinds LIFO
        exp_pool = ctx.enter_context(tc.tile_pool(name="expp", bufs=4))
        pexp_pool = ctx.enter_context(tc.tile_pool(name="pexpp", bufs=3))

        # ---- PSUM pool for the projection phase (closed afterwards) ----
        mm_cm = tc.tile_pool(name="mm", bufs=2, space="PSUM")
        mm_pool = mm_cm.__enter__()

        # ---- Q/K projections (transposed layout) ----
        for c in range(NC_D):
            for w_sb, b_sb, o_sb in ((wqT_sb, bq_sb, qT_sb),
                                     (wkT_sb, bk_sb, kT_sb)):
                ps = mm_pool.tile([128, S], F32, tag="mm")
                for kc in range(NC_D):
                    lhsT = w_sb[:, kc * D + c * 128: kc * D + (c + 1) * 128]
                    for sb in range(2):
                        nc.tensor.matmul(
                            ps[:, sb * 512:(sb + 1) * 512], lhsT,
                            hsT_sb[:, kc * S + sb * 512: kc * S + (sb + 1) * 512],
                            start=(kc == 0), stop=(kc == NC_D - 1))
                nc.vector.tensor_scalar_add(o_sb[:, c * S:(c + 1) * S],
                                            ps[:], b_sb[:, c:c + 1])

        # PSUM banks 4-7 (on top of mm's 0-3); closed before mm so the
        # pool stack unwinds LIFO, then reopened for pairs 1..5
        sc0_cm = tc.tile_pool(name="scp0", bufs=2, space="PSUM")
        scp = {"p": sc0_cm.__enter__()}

        def prefix_scores(c, pexp):
            sc_pool = scp["p"]
            psp = sc_pool.tile([128, S], F32, tag="sc", name=f"psp_{c}")
            for half in range(2):
                hp = half * 64
                for sb in range(2):
                    nc.tensor.matmul(
                        psp[hp:hp + 64, sb * 512:(sb + 1) * 512],
                        pkT_sb[hp:hp + 64, c * AT:(c + 1) * AT],
                        qT_sb[hp:hp + 64,
                              c * S + sb * 512: c * S + (sb + 1) * 512],
                        tile_position=(hp, hp))
            nc.scalar.activation(pexp[:], psp[:], AF.Exp, scale=SCALE)

        def scores_exp(c, exp_ab, pexp, ctx_mms=None, mid=()):
            sc_pool = scp["p"]
            """Scores + exp for pair c ([t,s] layout, 2 heads row-tiled);
            optionally interleaves ctx matmuls for chunk tci-1 to keep the
            PE dense, plus `mid` finish thunks for the previous pair.

            The prefix scores ride at tci 2 (before that chunk's own
            matmuls) — early enough that the ACT pexp drains its psum slot
            in rotation time, late enough that it never heads the queue."""
            for tci in range(NC_S):
                if tci == 2 and pexp is not None:
                    prefix_scores(c, pexp)
                for half in range(2):
                    hp = half * 64
                    st = sc_pool.tile([128, S], F32, tag="sc",
                                      name=f"st_{c}_{tci}_{half}")
                    lhsT = kT_sb[hp:hp + 64,
                                 c * S + tci * 128: c * S + (tci + 1) * 128]
                    for sb in range(2):
                        nc.tensor.matmul(
                            st[:, sb * 512:(sb + 1) * 512], lhsT,
                            qT_sb[hp:hp + 64,
                                  c * S + sb * 512: c * S + (sb + 1) * 512],
                            tile_position=(hp, 0))
                    # ACT/DVE alternate by half (swapped at tci 2, where
                    # the pexp op loads the ACT queue)
                    dst = exp_ab[half][:, tci * S:(tci + 1) * S]
                    if (half == 0) != (tci == 2):
                        nc.scalar.activation(dst, st[:], AF.Exp, scale=SCALE)
                    else:
                        nc.vector.tensor_scalar(
                            dst.bitcast(I16), st[:], EXP_A, EXP_B,
                            op0=ALU.mult, op1=ALU.add)
                if tci >= 1 and ctx_mms is not None:
                    for half in range(2):
                        ctx_mms(half, tci - 1)
                if 1 <= tci < 1 + len(mid):
                    mid[tci - 1]()

        # pair-0 scores start as soon as qT/kT chunk 0 exists, overlapping
        # the V/prompt projections below (ACT would otherwise sit idle).
        # The prefix part waits until pkT exists, in finish_pair(0).
        exp0 = [exp_pool.tile([128, NC_S * S], BF16, tag="exp",
                              name=f"exp_0_{i}") for i in range(2)]
        pexp0 = pexp_pool.tile([128, S], BF16, tag="pexp", name="pexp0")
        scores_exp(0, exp0, None)

        # ---- V projection (natural layout, augmented ones column) ----
        for sc in range(NC_S):
            ps = mm_pool.tile([128, S], F32, tag="mm")
            for kc in range(NC_D):
                lhsT = hsT_sb[:, kc * S + sc * 128: kc * S + (sc + 1) * 128]
                nc.tensor.matmul(ps[:, 0:512], lhsT,
                                 wvT_sb[:, kc * VW: kc * VW + 512],
                                 start=(kc == 0), stop=(kc == NC_D - 1))
                nc.tensor.matmul(ps[:, 512:VW], lhsT,
                                 wvT_sb[:, kc * VW + 512: (kc + 1) * VW],
                                 start=(kc == 0), stop=(kc == NC_D - 1))
            vt = proj.tile([128, VW], F32, tag="vtmp", name=f"vt{sc}",
                           bufs=2)
            nc.vector.tensor_add(vt[:], ps[:, 0:VW], bvaug_sb[:])
            nc.vector.tensor_scalar_mul(v_sb[:, sc * VW:(sc + 1) * VW],
                                        vt[:], emask_sb[:, sc:sc + 1])

        # ---- prompt K projection (transposed) ----
        for c in range(NC_D):
            ps = mm_pool.tile([128, S], F32, tag="mm")
            for kc in range(NC_D):
                nc.tensor.matmul(
                    ps[:, 0:AT],
                    wkT_sb[:, kc * D + c * 128: kc * D + (c + 1) * 128],
                    pT_sb[:, kc * AT:(kc + 1) * AT],
                    start=(kc == 0), stop=(kc == NC_D - 1))
            nc.vector.tensor_scalar_add(pkT_sb[:, c * AT:(c + 1) * AT],
                                        ps[:, 0:AT], bk_sb[:, c:c + 1])

        # ---- prompt V projection (natural, gate-scaled, duplicated) ----
        ps = mm_pool.tile([128, S], F32, tag="mm")
        for kc in range(NC_D):
            lhsT = pT_sb[:, kc * AT:(kc + 1) * AT]
            nc.tensor.matmul(ps[0:AT, 0:512], lhsT,
                             wvT_sb[:, kc * VW: kc * VW + 512],
                             start=(kc == 0), stop=(kc == NC_D - 1))
            nc.tensor.matmul(ps[0:AT, 512:VW], lhsT,
                             wvT_sb[:, kc * VW + 512: (kc + 1) * VW],
                             start=(kc == 0), stop=(kc == NC_D - 1))
        nc.vector.tensor_add(pvtmp_sb[:], ps[0:AT, 0:VW], bvaug_sb[0:AT, :])
        nc.vector.tensor_mul(pv_sb[0:AT, :], pvtmp_sb[:], gbc_sb[0:AT, :])
        nc.sync.dma_start(pv_sb[AT:128, :], pv_sb[0:AT, :])

        sc0_cm.__exit__(None, None, None)
        proj_cm.__exit__(None, None, None)
        mm_cm.__exit__(None, None, None)

        # ---- remaining attention pools (reuse the projection PSUM) ----
        scp["p"] = ctx.enter_context(
            tc.tile_pool(name="scp", bufs=2, space="PSUM"))
        ctx_pool = ctx.enter_context(
            tc.tile_pool(name="ctxp", bufs=2, space="PSUM"))
        norm_pool = ctx.enter_context(tc.tile_pool(name="normp", bufs=2))
        out_pool = ctx.enter_context(tc.tile_pool(name="outp", bufs=2))
        dscr_pool = ctx.enter_context(
            tc.tile_pool(name="dscr", bufs=2, space="DRAM"))

        def make_ctx_mms(c, cps_ab, exp_ab):
            def ctx_mms(half, tci):
                h = 2 * c + half
                lhsT = v_sb[:, tci * VW + h * 65: tci * VW + h * 65 + 65]
                for sb in range(2):
                    nc.tensor.matmul(
                        cps_ab[half][:, sb * 512:(sb + 1) * 512], lhsT,
                        exp_ab[half][:, tci * S + sb * 512:
                                     tci * S + (sb + 1) * 512],
                        start=(tci == 0), stop=(tci == NC_S - 1))
            return ctx_mms

        def finish_stage1(c, exp_ab, pexp, cps_ab):
            """Prefix ctx matmuls + psum-row staging at the block boundary.
            ACT does the evacuations (pe_ev first: the pps tiles sit in the
            score rotation and must drain within ~2 slots). The reciprocal
            and combine run as mid-block hooks inside the next pair's
            scores loop — the main ctx psum is normalized straight out of
            PSUM there, which frees its bank half a block before the
            next-next pair's accumulation needs it."""
            state = []
            for half in range(2):
                h = 2 * c + half
                hp = half * 64
                cps = cps_ab[half]
                pps = scp["p"].tile([128, S], F32, tag="sc",
                                    name=f"pps_{c}_{half}")
                for sb in range(2):
                    nc.tensor.matmul(
                        pps[0:65, sb * 512:(sb + 1) * 512],
                        pv_sb[hp:hp + 64, h * 65: h * 65 + 65],
                        pexp[hp:hp + 64, sb * 512:(sb + 1) * 512],
                        tile_position=(hp, 0))
                pe_ev = norm_pool.tile([65, S], F32, tag="pe_ev", bufs=4,
                                       name=f"pe_{c}_{half}")
                nc.scalar.copy(pe_ev[:], pps[0:65, :])
                state.append([h, cps, pe_ev, None, None])
            for half in range(2):
                h, cps, pe_ev, _, _ = state[half]
                den_c = norm_pool.tile([1, S], F32, tag="den", bufs=4,
                                       name=f"den_{c}_{half}")
                nc.scalar.copy(den_c[:], cps[64:65, :])
                dresh = norm_pool.tile([128, 16], F32, tag="dresh", bufs=4,
                                       name=f"dr_{c}_{half}")
                nc.sync.dma_start(dresh[:, 0:8], den_c[:])
                nc.sync.dma_start(dresh[:, 8:16], pe_ev[64:65, :])
                state[half][3] = dresh
            return state

        def finish_hooks(c, state):
            """Mid-block finish thunks for pair c, run inside the next
            pair's scores loop at tci 1..6."""
            def recips():
                for half in range(2):
                    h, cps, pe_ev, dresh, _ = state[half]
                    rrec = norm_pool.tile([128, 16], F32, tag="rrec",
                                          bufs=4, name=f"rr_{c}_{half}")
                    nc.vector.reciprocal(rrec[:], dresh[:])
                    r_d = dscr_pool.tile([1, 2 * S], F32, tag="rd", bufs=4,
                                         name=f"rd_{c}_{half}")
                    nc.sync.dma_start(r_d[0:1, 0:S], rrec[:, 0:8])
                    nc.sync.dma_start(r_d[0:1, S:2 * S], rrec[:, 8:16])
                    r_bc = norm_pool.tile([64, 2 * S], F32, tag="rbc",
                                          bufs=4, name=f"rbc_{c}_{half}")
                    r_src = bass.AP(r_d[:].tensor, r_d[:].offset,
                                    [[0, 64], [1, 2 * S]])
                    nc.sync.dma_start(r_bc[:], r_src)
                    state[half][4] = r_bc

            def muls(half):
                def t():
                    h, cps, pe_ev, _, r_bc = state[half]
                    ce_n = out_pool.tile([64, S], F32, tag="ce", bufs=2,
                                         name=f"ce_{h}")
                    nc.vector.tensor_mul(ce_n[:], cps[0:64, :],
                                         r_bc[:, 0:S])
                    pe_n = out_pool.tile([64, S], F32, tag="pe", bufs=2,
                                         name=f"pen_{h}")
                    nc.gpsimd.tensor_mul(pe_n[:], pe_ev[0:64, :],
                                         r_bc[:, S:2 * S])
                    state[half].append((ce_n, pe_n))
                return t

            def store(half):
                def t():
                    h = state[half][0]
                    ce_n, pe_n = state[half][5]
                    ot = out_pool.tile([64, S], F32, tag="ot", bufs=2,
                                       name=f"ot_{h}")
                    nc.gpsimd.tensor_add(ot[:], ce_n[:], pe_n[:])
                    nc.sync.dma_start(outT[h * 64:(h + 1) * 64, :], ot[:])
                return t

            return [recips, muls(0), muls(1), store(0), store(1)]

        # pair 0: ctx for the pre-computed exps, then the remaining pairs
        # with ctx interleaved behind their own score/exp stream
        cps0 = [ctx_pool.tile([65, S], F32, tag="ctx", name=f"cps_0_{i}")
                for i in range(2)]
        ctx0 = make_ctx_mms(0, cps0, exp0)
        prefix_scores(0, pexp0)
        for tci in range(NC_S):
            for half in range(2):
                ctx0(half, tci)
        hooks = finish_hooks(0, finish_stage1(0, exp0, pexp0, cps0))

        for c in range(1, PAIRS):
            exp_ab = [exp_pool.tile([128, NC_S * S], BF16, tag="exp",
                                    name=f"exp_{c}_{i}")
                      for i in range(2)]
            pexp = pexp_pool.tile([128, S], BF16, tag="pexp",
                                  name=f"pexp_{c}")
            cps_ab = [ctx_pool.tile([65, S], F32, tag="ctx",
                                    name=f"cps_{c}_{i}")
                      for i in range(2)]
            cmm = make_ctx_mms(c, cps_ab, exp_ab)
            scores_exp(c, exp_ab, pexp, ctx_mms=cmm, mid=hooks)
            for half in range(2):
                cmm(half, NC_S - 1)
            hooks = finish_hooks(c, finish_stage1(c, exp_ab, pexp, cps_ab))
        for t in hooks:
            t()


def _prep_inputs(hidden_states, prompt_tokens, gating_factor, attention_mask,
                 Wq, bq, Wk, bk, Wv, bv):
    bf = ml_dtypes.bfloat16
    hs = np.asarray(hidden_states, np.float32)
    mask = np.asarray(attention_mask, np.float32).reshape(B, S)
    wqT = np.ascontiguousarray(np.asarray(Wq, np.float32).T).astype(bf)
    wkT = np.ascontiguousarray(np.asarray(Wk, np.float32).T).astype(bf)
    # augmented WvT: [din, 780], col 65h+j = Wv.T[:, 64h+j], col 65h+64 = 0
    wvT_f = np.asarray(Wv, np.float32).T  # [din, dout]
    wvT_aug = np.zeros((D, VW), np.float32)
    idx = np.arange(D)
    aug_cols = (idx // DH) * (DH + 1) + (idx % DH)
    wvT_aug[:, aug_cols] = wvT_f
    wvT_aug = wvT_aug.astype(bf)
    bq_c = np.asarray(bq, np.float32).reshape(D, 1)
    bk_c = np.asarray(bk, np.float32).reshape(D, 1)
    bv_aug = np.zeros(VW, np.float32)
    bv_aug[aug_cols] = np.asarray(bv, np.float32)
    bv_aug[DH::DH + 1] = 1.0
    bvaug_bc = np.ascontiguousarray(
        np.broadcast_to(bv_aug, (128, VW)), np.float32)
    pT = np.ascontiguousarray(
        np.asarray(prompt_tokens, np.float32)[0].T).astype(bf)
    gat_row = np.repeat(
        np.asarray(gating_factor, np.float32).reshape(H), DH + 1)
    gat = np.ascontiguousarray(
        np.broadcast_to(gat_row, (128, VW)), np.float32)

    shared = dict(wqT=wqT, wkT=wkT, wvT=wvT_aug, bq=bq_c, bk=bk_c,
                  bvaug=bvaug_bc, promptT=pT, gating=gat)
    in_maps = []
    for b in range(B):
        m = dict(shared)
        m["hsT"] = np.ascontiguousarray(hs[b].T).astype(bf)
        m["mask"] = np.ascontiguousarray(mask[b].reshape(S, 1))
        in_maps.append(m)
    return in_maps


def kernel(**inputs):
    global LAST_RESULTS
    if "nc" not in _CACHE:
        _CACHE["nc"] = _build_nc()
    nc = _CACHE["nc"]
    in_maps = _prep_inputs(**inputs)
    res = None
    for attempt in range(3):
        try:
            res = run_bass_kernel_spmd(nc, in_maps, list(range(B)))
            break
        except ModuleNotFoundError:
            # BASS_TRACE set but this image lacks antenv.axon_hooks
            import os

            os.environ["BASS_NEVER_TRACE"] = "1"
            if attempt == 2:
                raise
        except Exception:
            # transient NRT_EXEC_UNIT_UNRECOVERABLE on a cold device has
            # been observed; a retry on the same session recovers
            if attempt == 2:
                raise
    LAST_RESULTS = res
    out = np.empty((B, S, D), np.float32)
    for b in range(B):
        out[b] = res.results[b]["outT"].T
    return out



# revision 42
# speedup vs baseline: 1.1458x; 1.1250x over previous
"""BertSelfAttention with gated prompt-prefix branch on 8 Trainium2 cores.

Sharding: data-parallel over batch (B=8 -> 1 batch element per core), no
collectives. Per core the pipeline runs in a transposed [feature, seq]
layout so softmax statistics ride through the matmuls:

  qT/kT = W @ hsT        [768, 1024] bf16
  v_aug = hs @ WvT_aug   [1024, 780], 65-col stride per head, col 65h+64
                         = ones (denominator column)
  scoresT_h = kh @ qh.T  [t, s], two heads row-tiled on the PE
  expT: half the tiles on ACT (true exp), half on DVE via a Schraudolph
        bit-trick (bf16_bits = int16(x*a + b)); a PE instruction whose
        semaphore wait is unsatisfied at decode resets the tensor
        engine's p-state ramp (3us continuous to reach 2.4GHz, else
        1.2GHz), so every drain must run well ahead of the score-psum
        rotation. Scores get a 3-deep [128,1024] rotation (6 banks) so
        each drain has ~2 chunk-periods of slack.
  ctxT_aug_h = v_aug_h.T @ expT_h, accumulated as two sequential
        s-halves in 2 psum banks; each half's denominator/reciprocal/
        normalize chain completes inside the block so the banks recycle.
  The ENTIRE prefix branch (prefix scores/exp/ctx/denominators) runs in
        the projection phase, which has engine slack — prefix tiles in
        the attention-score rotation would de-phase its lookahead.
  out_h = ctxT/denom + pctxT/pdenom  (DVE+GpSimd muls, GpSimd add)

Output is outT [768, 1024] fp32 per core; the host transposes/stacks.
"""

import numpy as np
import ml_dtypes

import concourse.bass as bass
import concourse.mybir as mybir
import concourse.tile as tile
from concourse.bass_utils import run_bass_kernel_spmd
from concourse.vector_clock import ScopedClock


class SplitDrainTileContext(tile.TileContext):
    """This walrus build rejects >2 sync waits on the kernel-tail Drain
    ("Too many sync wait commands"); split them across SP nops instead."""

    def _drain_and_barrier(self, tick_clock, wait_clock):
        probe = self.nc.sync.nop(nofuse=True, hint="drain_wait_split")
        wait_clock.add_sem_waits(
            probe.ins, ScopedClock({None: tick_clock.global_clock})
        )
        waits = list(probe.ins.sync_info.on_wait or [])
        if len(waits) > 1:
            probe.ins.sync_info.on_wait = waits[:1]
            for i in range(1, len(waits)):
                extra = self.nc.sync.nop(nofuse=True, hint="drain_wait_split")
                extra.ins.sync_info = mybir.SyncInfo(
                    on_wait=waits[i : i + 1], on_update=[]
                )
        drain_inst = self.nc.sync.drain()
        if drain_inst.ins.sync_info is not None:
            drain_inst.ins.sync_info.on_wait = []
        self.nc.all_engine_barrier()
        assert self.sems is not None
        popped = self.nc._tile_sem_poison_stack.pop()
        assert popped is self._sem_poison
        self.nc.clear_and_free_semaphores(list(self.sems.allocated().values()))
        self.nc.all_engine_barrier()

F32 = mybir.dt.float32
BF16 = mybir.dt.bfloat16
I16 = mybir.dt.int16
AF = mybir.ActivationFunctionType
ALU = mybir.AluOpType

H, DH, D = 12, 64, 768
S, AT, B = 1024, 64, 8
SCALE = 1.0 / np.sqrt(DH)
NC_D = D // 128  # 6 chunks over feature dim
NC_S = S // 128  # 8 chunks over sequence dim
PAIRS = H // 2  # 6 head pairs
VW = H * (DH + 1)  # 780: v with per-head ones column

# Schraudolph exp -> bf16 bits: bits = trunc(x*EXP_A + EXP_B); int16->bf16
# bitcast yields ~exp(SCALE*x) with ~1.8% rms error that washes out in the
# softmax-weighted context sums.
EXP_A = float(SCALE * 128.0 / np.log(2.0))
EXP_B = 16256.0 - 6.75

_CACHE = {}
LAST_RESULTS = None


def _split_sync_waits(nc, cap=1):
    """Walrus on this image allows very few sync-wait commands per
    instruction (tensor_scalar rejects 2). Hoist excess waits onto
    same-engine nops placed immediately before the instruction."""
    for bb in nc.main_func.blocks:
        cur = list(bb.instructions)
        out = []
        for inst in cur:
            si = inst.sync_info
            waits = list(si.on_wait) if si and si.on_wait else []
            if len(waits) > cap:
                for i in range(0, len(waits) - cap):
                    bi = nc.engines[inst.engine].nop(
                        nofuse=True, hint="wait_split")
                    popped = nc.cur_bb.bb.instructions.pop()
                    assert popped is bi.ins
                    bi.ins.sync_info = mybir.SyncInfo(
                        on_wait=waits[i : i + 1], on_update=[])
                    out.append(bi.ins)
                si.on_wait = waits[len(waits) - cap:]
            out.append(inst)
        bb.instructions[:] = out


def _build_nc():
    nc = bass.Bass()
    hsT = nc.dram_tensor("hsT", [D, S], BF16, kind="ExternalInput")
    wqT = nc.dram_tensor("wqT", [D, D], BF16, kind="ExternalInput")
    wkT = nc.dram_tensor("wkT", [D, D], BF16, kind="ExternalInput")
    wvT = nc.dram_tensor("wvT", [D, VW], BF16, kind="ExternalInput")
    bq = nc.dram_tensor("bq", [D, 1], F32, kind="ExternalInput")
    bk = nc.dram_tensor("bk", [D, 1], F32, kind="ExternalInput")
    bvaug = nc.dram_tensor("bvaug", [128, VW], F32, kind="ExternalInput")
    promptT = nc.dram_tensor("promptT", [D, AT], BF16, kind="ExternalInput")
    mask = nc.dram_tensor("mask", [S, 1], F32, kind="ExternalInput")
    gating = nc.dram_tensor("gating", [128, VW], F32, kind="ExternalInput")
    outT = nc.dram_tensor("outT", [D, S], F32, kind="ExternalOutput")

    with SplitDrainTileContext(nc) as tc:
        _emit(nc, tc, hsT, wqT, wkT, wvT, bq, bk, bvaug, promptT, mask,
              gating, outT)
    _split_sync_waits(nc)
    return nc


def _emit(nc, tc, hsT, wqT, wkT, wvT, bq, bk, bvaug, promptT, mask, gating,
          outT):
    from contextlib import ExitStack

    with ExitStack() as ctx:
        pers = ctx.enter_context(tc.tile_pool(name="pers", bufs=1))

        # ---- SBUF arrays that live into the attention phase ----
        mask_sb = pers.tile([128, NC_S], F32, tag="mask")
        emask_sb = pers.tile([128, NC_S], F32, tag="emask")
        qT_sb = pers.tile([128, NC_D * S], BF16, tag="qT")
        kT_sb = pers.tile([128, NC_D * S], BF16, tag="kT")
        v_sb = pers.tile([128, NC_S * VW], BF16, tag="v")
        pkT_sb = pers.tile([128, NC_D * AT], BF16, tag="pkT")
        pv_sb = pers.tile([128, VW], BF16, tag="pv")

        # ---- projection-phase-only arrays ----
        proj_cm = tc.tile_pool(name="proj", bufs=1, side="right")
        proj = proj_cm.__enter__()
        hsT_sb = proj.tile([128, NC_D * S], BF16, tag="hsT")
        wqT_sb = proj.tile([128, NC_D * D], BF16, tag="wqT")
        wkT_sb = proj.tile([128, NC_D * D], BF16, tag="wkT")
        wvT_sb = proj.tile([128, NC_D * VW], BF16, tag="wvT")
        pT_sb = proj.tile([128, NC_D * AT], BF16, tag="pT")
        bq_sb = proj.tile([128, NC_D], F32, tag="bq")
        bk_sb = proj.tile([128, NC_D], F32, tag="bk")
        bvaug_sb = proj.tile([128, VW], F32, tag="bvaug")
        graw_sb = proj.tile([128, VW], F32, tag="graw")
        gbc_sb = proj.tile([128, VW], F32, tag="gbc")
        pvtmp_sb = proj.tile([64, VW], F32, tag="pvtmp")

        for src, dst, w in ((wqT, wqT_sb, D), (hsT, hsT_sb, S),
                            (wkT, wkT_sb, D), (wvT, wvT_sb, VW),
                            (promptT, pT_sb, AT)):
            nc.sync.dma_start(
                dst[:].rearrange("p (c s) -> p c s", s=w),
                src[:, :].rearrange("(c p) s -> p c s", p=128))
        nc.sync.dma_start(bq_sb[:], bq.rearrange("(c p) 1 -> p c", p=128))
        nc.sync.dma_start(bk_sb[:], bk.rearrange("(c p) 1 -> p c", p=128))
        nc.sync.dma_start(mask_sb[:], mask.rearrange("(c p) 1 -> p c", p=128))
        nc.sync.dma_start(bvaug_sb[:], bvaug[:])
        nc.sync.dma_start(graw_sb[:], gating[:])
        # tanh, then force the ones-column slots back to 1.0
        nc.scalar.activation(gbc_sb[:], graw_sb[:], AF.Tanh)
        ones_slots = gbc_sb[:, :].rearrange(
            "p (h e) -> p h e", h=H)[:, :, DH:DH + 1]
        nc.vector.memset(ones_slots, 1.0)
        # e^mask folded into the V rows (incl. ones column)
        nc.scalar.activation(emask_sb[:], mask_sb[:], AF.Exp)

        # SBUF pools that outlive the projection phase
        exp_pool = ctx.enter_context(tc.tile_pool(name="expp", bufs=3))
        pexp_pool = ctx.enter_context(tc.tile_pool(name="pexpp", bufs=2))
        pep_pool = ctx.enter_context(tc.tile_pool(name="pep", bufs=1))
        dscr_pool = ctx.enter_context(
            tc.tile_pool(name="dscr", bufs=2, space="DRAM"))

        # ---- PSUM pools for the projection phase ----
        mm_cm = tc.tile_pool(name="mm", bufs=2, space="PSUM")
        mm_pool = mm_cm.__enter__()
        sc0_cm = tc.tile_pool(name="scp0", bufs=2, space="PSUM")
        scp = {"p": sc0_cm.__enter__()}

        exps = {0: [exp_pool.tile([128, NC_S * S], BF16, tag="exp",
                                  name=f"exp_0_{i}") for i in range(2)]}

        def scores_tci(c, tci, exp_ab):
            """Scores + exp for (pair c, chunk tci), 2 heads row-tiled;
            h0 exp'd on ACT, h1 via Schraudolph on the DVE."""
            for half in range(2):
                hp = half * 64
                st = scp["p"].tile([128, S], F32, tag="sc",
                                   name=f"st_{c}_{tci}_{half}")
                lhsT = kT_sb[hp:hp + 64,
                             c * S + tci * 128: c * S + (tci + 1) * 128]
                for sb in range(2):
                    nc.tensor.matmul(
                        st[:, sb * 512:(sb + 1) * 512], lhsT,
                        qT_sb[hp:hp + 64,
                              c * S + sb * 512: c * S + (sb + 1) * 512],
                        tile_position=(hp, 0))
                dst = exp_ab[half][:, tci * S:(tci + 1) * S]
                if half == 0:
                    nc.scalar.activation(dst, st[:], AF.Exp, scale=SCALE)
                else:
                    nc.vector.tensor_scalar(
                        dst.bitcast(I16), st[:], EXP_A, EXP_B,
                        op0=ALU.mult, op1=ALU.add)

        def qk_chain(c, w_sb, b_sb, o_sb):
            ps = mm_pool.tile([128, S], F32, tag="mm")
            for kc in range(NC_D):
                lhsT = w_sb[:, kc * D + c * 128: kc * D + (c + 1) * 128]
                for sb in range(2):
                    nc.tensor.matmul(
                        ps[:, sb * 512:(sb + 1) * 512], lhsT,
                        hsT_sb[:, kc * S + sb * 512: kc * S + (sb + 1) * 512],
                        start=(kc == 0), stop=(kc == NC_D - 1))
            nc.vector.tensor_scalar_add(o_sb[:, c * S:(c + 1) * S],
                                        ps[:], b_sb[:, c:c + 1])

        def v_chunk(sc):
            ps = mm_pool.tile([128, S], F32, tag="mm")
            for kc in range(NC_D):
                lhsT = hsT_sb[:, kc * S + sc * 128: kc * S + (sc + 1) * 128]
                nc.tensor.matmul(ps[:, 0:512], lhsT,
                                 wvT_sb[:, kc * VW: kc * VW + 512],
                                 start=(kc == 0), stop=(kc == NC_D - 1))
                nc.tensor.matmul(ps[:, 512:VW], lhsT,
                                 wvT_sb[:, kc * VW + 512: (kc + 1) * VW],
                                 start=(kc == 0), stop=(kc == NC_D - 1))
            vt = proj.tile([128, VW], F32, tag="vtmp", name=f"vt{sc}",
                           bufs=2)
            nc.vector.tensor_add(vt[:], ps[:, 0:VW], bvaug_sb[:])
            nc.vector.tensor_scalar_mul(v_sb[:, sc * VW:(sc + 1) * VW],
                                        vt[:], emask_sb[:, sc:sc + 1])

        # ---- QK proj chunk 0, then pair-0 scores with QK-chain spacers ----
        qk_chain(0, wqT_sb, bq_sb, qT_sb)
        qk_chain(0, wkT_sb, bk_sb, kT_sb)
        qk_spacers = []
        for c in range(1, NC_D):
            qk_spacers.append(lambda c=c: qk_chain(c, wqT_sb, bq_sb, qT_sb))
            qk_spacers.append(lambda c=c: qk_chain(c, wkT_sb, bk_sb, kT_sb))
        for tci in range(NC_S):
            scores_tci(0, tci, exps[0])
            n = 2 if tci < 2 else 1
            for _ in range(n):
                if qk_spacers:
                    qk_spacers.pop(0)()

        # ---- prompt K projection (transposed) ----
        for c in range(NC_D):
            ps = mm_pool.tile([128, S], F32, tag="mm")
            for kc in range(NC_D):
                nc.tensor.matmul(
                    ps[:, 0:AT],
                    wkT_sb[:, kc * D + c * 128: kc * D + (c + 1) * 128],
                    pT_sb[:, kc * AT:(kc + 1) * AT],
                    start=(kc == 0), stop=(kc == NC_D - 1))
            nc.vector.tensor_scalar_add(pkT_sb[:, c * AT:(c + 1) * AT],
                                        ps[:, 0:AT], bk_sb[:, c:c + 1])

        # ---- prompt V projection (natural, gate-scaled, duplicated) ----
        ps = mm_pool.tile([128, S], F32, tag="mm")
        for kc in range(NC_D):
            lhsT = pT_sb[:, kc * AT:(kc + 1) * AT]
            nc.tensor.matmul(ps[0:AT, 0:512], lhsT,
                             wvT_sb[:, kc * VW: kc * VW + 512],
                             start=(kc == 0), stop=(kc == NC_D - 1))
            nc.tensor.matmul(ps[0:AT, 512:VW], lhsT,
                             wvT_sb[:, kc * VW + 512: (kc + 1) * VW],
                             start=(kc == 0), stop=(kc == NC_D - 1))
        nc.vector.tensor_add(pvtmp_sb[:], ps[0:AT, 0:VW], bvaug_sb[0:AT, :])
        nc.vector.tensor_mul(pv_sb[0:AT, :], pvtmp_sb[:], gbc_sb[0:AT, :])
        nc.sync.dma_start(pv_sb[AT:128, :], pv_sb[0:AT, :])

        # ---- entire prefix branch, V chunks as PE spacers ----
        # per pair: prefix scores -> exp -> prefix ctx (ones column gives
        # the prefix denominator) -> bf16 evacuation + reciprocal chain to
        # DRAM (broadcast back during the attention phase).
        pe_ev = {}
        rdp = {}
        vq = list(range(NC_S))
        for c in range(PAIRS):
            if vq:
                v_chunk(vq.pop(0))
            psp = scp["p"].tile([128, S], F32, tag="sc", name=f"psp{c}")
            for half in range(2):
                hp = half * 64
                for sb in range(2):
                    nc.tensor.matmul(
                        psp[hp:hp + 64, sb * 512:(sb + 1) * 512],
                        pkT_sb[hp:hp + 64, c * AT:(c + 1) * AT],
                        qT_sb[hp:hp + 64,
                              c * S + sb * 512: c * S + (sb + 1) * 512],
                        tile_position=(hp, hp))
            pexp = pexp_pool.tile([128, S], BF16, tag="pexp",
                                  name=f"pexp{c}")
            nc.scalar.activation(pexp[:], psp[:], AF.Exp, scale=SCALE)
            if vq:
                v_chunk(vq.pop(0))
            dresh = proj.tile([128, 16], BF16, tag="drp", bufs=3,
                              name=f"drp{c}")
            for half in range(2):
                h = 2 * c + half
                hp = half * 64
                pps = scp["p"].tile([128, S], F32, tag="sc",
                                    name=f"pps{c}_{half}")
                for sb in range(2):
                    nc.tensor.matmul(
                        pps[0:65, sb * 512:(sb + 1) * 512],
                        pv_sb[hp:hp + 64, h * 65: h * 65 + 65],
                        pexp[hp:hp + 64, sb * 512:(sb + 1) * 512],
                        tile_position=(hp, 0))
                ev = pep_pool.tile([65, S], BF16, tag=f"pe{c}_{half}")
                with nc.allow_low_precision(
                        reason="prefix ctx to bf16: 0.4%% on the gated "
                               "prefix branch only"):
                    nc.scalar.copy(ev[:], pps[0:65, :])
                pe_ev[(c, half)] = ev
                nc.sync.dma_start(dresh[:, half * 8:(half + 1) * 8],
                                  ev[64:65, :])
            rrec = proj.tile([128, 16], BF16, tag="rrp", bufs=3,
                             name=f"rrp{c}")
            with nc.allow_low_precision(
                    reason="prefix denominator reciprocal in bf16"):
                nc.vector.reciprocal(rrec[:], dresh[:])
            rd = dscr_pool.tile([1, 2 * S], BF16, tag=f"rdp{c}", bufs=1,
                                name=f"rdp{c}")
            nc.sync.dma_start(rd[0:1, 0:S], rrec[:, 0:8])
            nc.sync.dma_start(rd[0:1, S:2 * S], rrec[:, 8:16])
            rdp[c] = rd
        while vq:
            v_chunk(vq.pop(0))

        sc0_cm.__exit__(None, None, None)
        proj_cm.__exit__(None, None, None)
        mm_cm.__exit__(None, None, None)

        # ---- attention-phase pools ----
        # banks 0-5: score rotation (3 x [128,1024]); banks 6-7: the two
        # ctx accumulators ([65,512] per head, s-halves sequential).
        scp["p"] = ctx.enter_context(
            tc.tile_pool(name="scp", bufs=3, space="PSUM"))
        ctx_pool = ctx.enter_context(
            tc.tile_pool(name="ctxp", bufs=1, space="PSUM"))
        norm_pool = ctx.enter_context(tc.tile_pool(name="normp", bufs=2))
        out_pool = ctx.enter_context(tc.tile_pool(name="outp", bufs=2))

        pend = {}

        def ctx_mm(p, half, k, sb):
            h = 2 * p + half
            lhsT = v_sb[:, k * VW + h * 65: k * VW + h * 65 + 65]
            nc.tensor.matmul(
                pend[p]["cps"][sb][half][0:65, :], lhsT,
                exps[p][half][:, k * S + sb * 512: k * S + (sb + 1) * 512],
                start=(k == 0), stop=(k == NC_S - 1))

        def ctx_alloc(p, sb):
            pend[p]["cps"][sb] = [
                ctx_pool.tile([65, 512], F32, tag="cA",
                              name=f"cps{p}_{sb}_0"),
                ctx_pool.tile([65, 512], F32, tag="cB",
                              name=f"cps{p}_{sb}_1")]

        def dens_sb0(p):
            """Evacuate the sb0 denominator rows right after the last sb0
            accumulation matmul (ACT: DMA cannot read PSUM)."""
            st = pend[p]
            st["dresh"] = norm_pool.tile([128, 8], F32, tag="dresh", bufs=4,
                                         name=f"dr{p}")
            st["rd"] = []
            st["rbc"] = []
            st["ce_n"] = []
            for half in range(2):
                den = norm_pool.tile([1, 512], F32, tag="den", bufs=4,
                                     name=f"den{p}_{half}")
                nc.scalar.copy(den[:], st["cps"][0][half][64:65, :])
                nc.sync.dma_start(st["dresh"][:, 4 * half: 4 * half + 4],
                                  den[0:1, :])

        def recips_sb0(p):
            """sb0 reciprocal + broadcast, then normalize sb0 straight out
            of PSUM on the DVE (frees the ctx banks for sb1)."""
            st = pend[p]
            rrec = norm_pool.tile([128, 8], F32, tag="rrec", bufs=4,
                                  name=f"rr{p}")
            nc.vector.reciprocal(rrec[:, 0:4], st["dresh"][:, 0:4])
            nc.vector.reciprocal(rrec[:, 4:8], st["dresh"][:, 4:8])
            for half in range(2):
                rd = dscr_pool.tile([1, S], F32, tag="rdm", bufs=4,
                                    name=f"rd{p}_{half}")
                nc.sync.dma_start(rd[0:1, 0:512],
                                  rrec[:, 4 * half:4 * half + 4])
                rbc = norm_pool.tile([64, S], F32, tag="rbc", bufs=4,
                                     name=f"rbc{p}_{half}")
                r_src = bass.AP(rd[:].tensor, rd[:].offset, [[0, 64], [1, 512]])
                nc.sync.dma_start(rbc[:, 0:512], r_src)
                st["rd"].append(rd)
                st["rbc"].append(rbc)
                ce_n = out_pool.tile([64, S], F32, tag="ce", bufs=4,
                                     name=f"ce{p}_{half}")
                nc.vector.tensor_mul(ce_n[:, 0:512],
                                     st["cps"][0][half][0:64, :],
                                     rbc[:, 0:512])
                st["ce_n"].append(ce_n)

        def evac_sb1(p):
            """Copy the sb1 accumulators (with denominator rows) to SBUF on
            ACT so the banks free before the next block's sb0 needs them;
            the normalization happens next block on GpSimd."""
            st = pend[p]
            st["ce_ev"] = []
            for half in range(2):
                ev = norm_pool.tile([65, 512], F32, tag="cev", bufs=4,
                                    name=f"cev{p}_{half}")
                nc.scalar.copy(ev[:], st["cps"][1][half][0:65, :])
                st["ce_ev"].append(ev)
                nc.sync.dma_start(st["dresh"][:, 4 * half:4 * half + 2],
                                  ev[64:65, 0:256])
                nc.sync.dma_start(st["dresh"][:, 4 * half + 2:4 * half + 4],
                                  ev[64:65, 256:512])

        def finish_q(q, step):
            """Combine steps for pair q (two blocks behind): sb1 recip +
            normalize from the SBUF copy, prefix normalize, add, store."""
            st = pend[q]
            if step == 0:
                rrec = norm_pool.tile([128, 8], F32, tag="rrec", bufs=4,
                                      name=f"rr2{q}")
                nc.vector.reciprocal(rrec[:, 0:4], st["dresh"][:, 0:4])
                nc.vector.reciprocal(rrec[:, 4:8], st["dresh"][:, 4:8])
                for half in range(2):
                    rd = st["rd"][half]
                    nc.sync.dma_start(rd[0:1, 512:1024],
                                      rrec[:, 4 * half:4 * half + 4])
                    rbc = st["rbc"][half]
                    r_src = bass.AP(rd[:].tensor, rd[:].offset + 512,
                                    [[0, 64], [1, 512]])
                    nc.sync.dma_start(rbc[:, 512:1024], r_src)
                rbp = norm_pool.tile([64, 2 * S], BF16, tag="rbp", bufs=4,
                                     name=f"rbp{q}")
                rd_p = rdp[q]
                r_src = bass.AP(rd_p[:].tensor, rd_p[:].offset,
                                [[0, 64], [1, 2 * S]])
                nc.sync.dma_start(rbp[:], r_src)
                st["rbp"] = rbp
                for half in range(2):
                    nc.gpsimd.tensor_mul(st["ce_n"][half][:, 512:1024],
                                         st["ce_ev"][half][0:64, :],
                                         st["rbc"][half][:, 512:1024])
            else:
                half = step - 1
                h = 2 * q + half
                pe_n = out_pool.tile([64, S], F32, tag="pe", bufs=2,
                                     name=f"pen{h}")
                nc.gpsimd.tensor_mul(
                    pe_n[:], pe_ev[(q, half)][0:64, :],
                    st["rbp"][:, half * S:(half + 1) * S])
                ot = out_pool.tile([64, S], F32, tag="ot", bufs=2,
                                   name=f"ot{h}")
                nc.gpsimd.tensor_add(ot[:], st["ce_n"][half][:], pe_n[:])
                nc.sync.dma_start(outT[h * 64:(h + 1) * 64, :], ot[:])

        def attention_block(c):
            """Scores for pair c + full ctx/denominator pipeline for pair
            c-1 + combine/store for pair c-2."""
            p, q = c - 1, c - 2
            pend[p] = {"cps": [None, None]}
            for tci in range(NC_S):
                if tci == 0 and q >= 0:
                    finish_q(q, 0)
                if tci == 4:
                    # before this chunk's scores: the DVE chain that frees
                    # the ctx banks for the sb1 phase
                    recips_sb0(p)
                scores_tci(c, tci, exps[c])
                if tci == 1:
                    ctx_alloc(p, 0)
                if 1 <= tci <= 3:
                    for k in range(3 * (tci - 1), min(3 * tci, 8)):
                        ctx_mm(p, 0, k, 0)
                        ctx_mm(p, 1, k, 0)
                if tci == 3:
                    dens_sb0(p)
                elif tci == 5:
                    ctx_alloc(p, 1)
                    for k in range(4):
                        ctx_mm(p, 0, k, 1)
                        ctx_mm(p, 1, k, 1)
                elif tci == 6:
                    for k in range(4, 8):
                        ctx_mm(p, 0, k, 1)
                        ctx_mm(p, 1, k, 1)
                    evac_sb1(p)
                elif tci == 7 and q >= 0:
                    finish_q(q, 1)
            if q >= 0:
                finish_q(q, 2)

        for c in range(1, PAIRS):
            exps[c] = [exp_pool.tile([128, NC_S * S], BF16, tag="exp",
                                     name=f"exp_{c}_{i}") for i in range(2)]
            attention_block(c)

        # ---- tail: ctx(5) + remaining finishes ----
        p, q = PAIRS - 1, PAIRS - 2
        pend[p] = {"cps": [None, None]}
        finish_q(q, 0)
        ctx_alloc(p, 0)
        for k in range(8):
            ctx_mm(p, 0, k, 0)
            ctx_mm(p, 1, k, 0)
            if k == 3:
                finish_q(q, 1)
        dens_sb0(p)
        finish_q(q, 2)
        recips_sb0(p)
        ctx_alloc(p, 1)
        for k in range(8):
            ctx_mm(p, 0, k, 1)
            ctx_mm(p, 1, k, 1)
        evac_sb1(p)
        finish_q(p, 0)
        finish_q(p, 1)
        finish_q(p, 2)


def _prep_inputs(hidden_states, prompt_tokens, gating_factor, attention_mask,
                 Wq, bq, Wk, bk, Wv, bv):
    bf = ml_dtypes.bfloat16
    hs = np.asarray(hidden_states, np.float32)
    mask = np.asarray(attention_mask, np.float32).reshape(B, S)
    wqT = np.ascontiguousarray(np.asarray(Wq, np.float32).T).astype(bf)
    wkT = np.ascontiguousarray(np.asarray(Wk, np.float32).T).astype(bf)
    # augmented WvT: [din, 780], col 65h+j = Wv.T[:, 64h+j], col 65h+64 = 0
    wvT_f = np.asarray(Wv, np.float32).T  # [din, dout]
    wvT_aug = np.zeros((D, VW), np.float32)
    idx = np.arange(D)
    aug_cols = (idx // DH) * (DH + 1) + (idx % DH)
    wvT_aug[:, aug_cols] = wvT_f
    wvT_aug = wvT_aug.astype(bf)
    bq_c = np.asarray(bq, np.float32).reshape(D, 1)
    bk_c = np.asarray(bk, np.float32).reshape(D, 1)
    bv_aug = np.zeros(VW, np.float32)
    bv_aug[aug_cols] = np.asarray(bv, np.float32)
    bv_aug[DH::DH + 1] = 1.0
    bvaug_bc = np.ascontiguousarray(
        np.broadcast_to(bv_aug, (128, VW)), np.float32)
    pT = np.ascontiguousarray(
        np.asarray(prompt_tokens, np.float32)[0].T).astype(bf)
    gat_row = np.repeat(
        np.asarray(gating_factor, np.float32).reshape(H), DH + 1)
    gat = np.ascontiguousarray(
        np.broadcast_to(gat_row, (128, VW)), np.float32)

    shared = dict(wqT=wqT, wkT=wkT, wvT=wvT_aug, bq=bq_c, bk=bk_c,
                  bvaug=bvaug_bc, promptT=pT, gating=gat)
    in_maps = []
    for b in range(B):
        m = dict(shared)
        m["hsT"] = np.ascontiguousarray(hs[b].T).astype(bf)
        m["mask"] = np.ascontiguousarray(mask[b].reshape(S, 1))
        in_maps.append(m)
    return in_maps


def kernel(**inputs):
    global LAST_RESULTS
    if "nc" not in _CACHE:
        _CACHE["nc"] = _build_nc()
    nc = _CACHE["nc"]
    in_maps = _prep_inputs(**inputs)
    res = None
    for attempt in range(3):
        try:
            res = run_bass_kernel_spmd(nc, in_maps, list(range(B)))
            break
        except ModuleNotFoundError:
            import os

            os.environ["BASS_NEVER_TRACE"] = "1"
            if attempt == 2:
                raise
        except Exception:
            if attempt == 2:
                raise
    LAST_RESULTS = res
    out = np.empty((B, S, D), np.float32)
    for b in range(B):
        out[b] = res.results[b]["outT"].T
    return out


# revision 43
# speedup vs baseline: 1.3734x; 1.1986x over previous
"""BertSelfAttention with gated prompt-prefix branch on 8 Trainium2 cores.

Sharding: data-parallel over batch (B=8 -> 1 batch element per core), no
collectives. Per core the pipeline runs in a transposed [feature, seq]
layout so softmax statistics ride through the matmuls:

  qT/kT = W @ hsT        [768, 1024] bf16
  v_aug = hs @ WvT_aug   [1024, 780], 65-col stride per head, col 65h+64
                         = ones (denominator column)
  scoresT_h = kh @ qh.T  [t, s], two heads row-tiled on the PE
  expT: half the tiles on ACT (true exp), half on DVE via a Schraudolph
        bit-trick (bf16_bits = int16(x*a + b)); a PE instruction whose
        semaphore wait is unsatisfied at decode resets the tensor
        engine's p-state ramp (3us continuous to reach 2.4GHz, else
        1.2GHz), so every drain must run well ahead of the score-psum
        rotation. Scores get a 3-deep [128,1024] rotation (6 banks) so
        each drain has ~2 chunk-periods of slack.
  ctxT_aug_h = v_aug_h.T @ expT_h, accumulated as two sequential
        s-halves in 2 psum banks; each half's denominator/reciprocal/
        normalize chain completes inside the block so the banks recycle.
  The ENTIRE prefix branch (prefix scores/exp/ctx/denominators) runs in
        the projection phase, which has engine slack — prefix tiles in
        the attention-score rotation would de-phase its lookahead.
  out_h = ctxT/denom + pctxT/pdenom  (DVE+GpSimd muls, GpSimd add)

Output is outT [768, 1024] fp32 per core; the host transposes/stacks.
"""

import numpy as np
import ml_dtypes

import concourse.bass as bass
import concourse.mybir as mybir
import concourse.tile as tile
from concourse.bass_utils import run_bass_kernel_spmd
from concourse.vector_clock import ScopedClock


class SplitDrainTileContext(tile.TileContext):
    """This walrus build rejects >2 sync waits on the kernel-tail Drain
    ("Too many sync wait commands"); split them across SP nops instead."""

    def _drain_and_barrier(self, tick_clock, wait_clock):
        probe = self.nc.sync.nop(nofuse=True, hint="drain_wait_split")
        wait_clock.add_sem_waits(
            probe.ins, ScopedClock({None: tick_clock.global_clock})
        )
        waits = list(probe.ins.sync_info.on_wait or [])
        if len(waits) > 1:
            probe.ins.sync_info.on_wait = waits[:1]
            for i in range(1, len(waits)):
                extra = self.nc.sync.nop(nofuse=True, hint="drain_wait_split")
                extra.ins.sync_info = mybir.SyncInfo(
                    on_wait=waits[i : i + 1], on_update=[]
                )
        drain_inst = self.nc.sync.drain()
        if drain_inst.ins.sync_info is not None:
            drain_inst.ins.sync_info.on_wait = []
        self.nc.all_engine_barrier()
        assert self.sems is not None
        popped = self.nc._tile_sem_poison_stack.pop()
        assert popped is self._sem_poison
        self.nc.clear_and_free_semaphores(list(self.sems.allocated().values()))
        self.nc.all_engine_barrier()

F32 = mybir.dt.float32
BF16 = mybir.dt.bfloat16
I16 = mybir.dt.int16
AF = mybir.ActivationFunctionType
ALU = mybir.AluOpType

H, DH, D = 12, 64, 768
S, AT, B = 1024, 64, 8
SCALE = 1.0 / np.sqrt(DH)
NC_D = D // 128  # 6 chunks over feature dim
NC_S = S // 128  # 8 chunks over sequence dim
PAIRS = H // 2  # 6 head pairs
VW = H * (DH + 1)  # 780: v with per-head ones column

# Schraudolph exp -> bf16 bits: bits = trunc(x*EXP_A + EXP_B); int16->bf16
# bitcast yields ~exp(SCALE*x) with ~1.8% rms error that washes out in the
# softmax-weighted context sums.
EXP_A = float(SCALE * 128.0 / np.log(2.0))
EXP_B = 16256.0 - 6.75

_CACHE = {}
LAST_RESULTS = None


def _split_sync_waits(nc, cap=1):
    """Walrus on this image allows very few sync-wait commands per
    instruction (tensor_scalar rejects 2). Hoist excess waits onto
    same-engine nops placed immediately before the instruction."""
    for bb in nc.main_func.blocks:
        cur = list(bb.instructions)
        out = []
        for inst in cur:
            si = inst.sync_info
            waits = list(si.on_wait) if si and si.on_wait else []
            if len(waits) > cap:
                for i in range(0, len(waits) - cap):
                    bi = nc.engines[inst.engine].nop(
                        nofuse=True, hint="wait_split")
                    popped = nc.cur_bb.bb.instructions.pop()
                    assert popped is bi.ins
                    bi.ins.sync_info = mybir.SyncInfo(
                        on_wait=waits[i : i + 1], on_update=[])
                    out.append(bi.ins)
                si.on_wait = waits[len(waits) - cap:]
            out.append(inst)
        bb.instructions[:] = out


def _build_nc():
    nc = bass.Bass()
    hsT = nc.dram_tensor("hsT", [D, S], BF16, kind="ExternalInput")
    wqT = nc.dram_tensor("wqT", [D, D], BF16, kind="ExternalInput")
    wkT = nc.dram_tensor("wkT", [D, D], BF16, kind="ExternalInput")
    wvT = nc.dram_tensor("wvT", [D, VW], BF16, kind="ExternalInput")
    bq = nc.dram_tensor("bq", [D, 1], F32, kind="ExternalInput")
    bk = nc.dram_tensor("bk", [D, 1], F32, kind="ExternalInput")
    bvaug = nc.dram_tensor("bvaug", [128, VW], F32, kind="ExternalInput")
    promptT = nc.dram_tensor("promptT", [D, AT], BF16, kind="ExternalInput")
    mask = nc.dram_tensor("mask", [S, 1], F32, kind="ExternalInput")
    gating = nc.dram_tensor("gating", [128, VW], F32, kind="ExternalInput")
    outT = nc.dram_tensor("outT", [D, S], F32, kind="ExternalOutput")

    with SplitDrainTileContext(nc) as tc:
        _emit(nc, tc, hsT, wqT, wkT, wvT, bq, bk, bvaug, promptT, mask,
              gating, outT)
    _split_sync_waits(nc)
    return nc


def _emit(nc, tc, hsT, wqT, wkT, wvT, bq, bk, bvaug, promptT, mask, gating,
          outT):
    from contextlib import ExitStack

    with ExitStack() as ctx:
        pers = ctx.enter_context(tc.tile_pool(name="pers", bufs=1))

        # ---- SBUF arrays that live into the attention phase ----
        mask_sb = pers.tile([128, NC_S], F32, tag="mask")
        emask_sb = pers.tile([128, NC_S], F32, tag="emask")
        qT_sb = pers.tile([128, NC_D * S], BF16, tag="qT")
        kT_sb = pers.tile([128, NC_D * S], BF16, tag="kT")
        v_sb = pers.tile([128, NC_S * VW], BF16, tag="v")
        pkT_sb = pers.tile([128, NC_D * AT], BF16, tag="pkT")
        pv_sb = pers.tile([128, VW], BF16, tag="pv")

        # ---- projection-phase-only arrays ----
        proj_cm = tc.tile_pool(name="proj", bufs=1, side="right")
        proj = proj_cm.__enter__()
        hsT_sb = proj.tile([128, NC_D * S], BF16, tag="hsT")
        wqT_sb = proj.tile([128, NC_D * D], BF16, tag="wqT")
        wkT_sb = proj.tile([128, NC_D * D], BF16, tag="wkT")
        wvT_sb = proj.tile([128, NC_D * VW], BF16, tag="wvT")
        pT_sb = proj.tile([128, NC_D * AT], BF16, tag="pT")
        bq_sb = proj.tile([128, NC_D], F32, tag="bq")
        bk_sb = proj.tile([128, NC_D], F32, tag="bk")
        bvaug_sb = proj.tile([128, VW], F32, tag="bvaug")
        graw_sb = proj.tile([128, VW], F32, tag="graw")
        gbc_sb = proj.tile([128, VW], F32, tag="gbc")
        pvtmp_sb = proj.tile([64, VW], F32, tag="pvtmp")

        for src, dst, w in ((wqT, wqT_sb, D), (hsT, hsT_sb, S),
                            (wkT, wkT_sb, D), (wvT, wvT_sb, VW),
                            (promptT, pT_sb, AT)):
            nc.sync.dma_start(
                dst[:].rearrange("p (c s) -> p c s", s=w),
                src[:, :].rearrange("(c p) s -> p c s", p=128))
        nc.sync.dma_start(bq_sb[:], bq.rearrange("(c p) 1 -> p c", p=128))
        nc.sync.dma_start(bk_sb[:], bk.rearrange("(c p) 1 -> p c", p=128))
        nc.sync.dma_start(mask_sb[:], mask.rearrange("(c p) 1 -> p c", p=128))
        nc.sync.dma_start(bvaug_sb[:], bvaug[:])
        nc.sync.dma_start(graw_sb[:], gating[:])
        # tanh, then force the ones-column slots back to 1.0
        nc.scalar.activation(gbc_sb[:], graw_sb[:], AF.Tanh)
        ones_slots = gbc_sb[:, :].rearrange(
            "p (h e) -> p h e", h=H)[:, :, DH:DH + 1]
        nc.vector.memset(ones_slots, 1.0)
        # e^mask folded into the V rows (incl. ones column)
        nc.scalar.activation(emask_sb[:], mask_sb[:], AF.Exp)

        # SBUF pools that outlive the projection phase
        exp_pool = ctx.enter_context(tc.tile_pool(name="expp", bufs=3))
        pexp_pool = ctx.enter_context(tc.tile_pool(name="pexpp", bufs=2))
        pep_pool = ctx.enter_context(tc.tile_pool(name="pep", bufs=1))
        dscr_pool = ctx.enter_context(
            tc.tile_pool(name="dscr", bufs=2, space="DRAM"))

        # ---- PSUM pools for the projection phase ----
        mm_cm = tc.tile_pool(name="mm", bufs=2, space="PSUM")
        mm_pool = mm_cm.__enter__()
        sc0_cm = tc.tile_pool(name="scp0", bufs=2, space="PSUM")
        scp = {"p": sc0_cm.__enter__()}

        exps = {0: [exp_pool.tile([128, NC_S * S], BF16, tag="exp",
                                  name=f"exp_0_{i}") for i in range(2)]}

        def scores_tci(c, tci, exp_ab):
            """Scores + exp for (pair c, chunk tci), 2 heads row-tiled;
            h0 exp'd on ACT, h1 via Schraudolph on the DVE."""
            for half in range(2):
                hp = half * 64
                st = scp["p"].tile([128, S], F32, tag="sc",
                                   name=f"st_{c}_{tci}_{half}")
                lhsT = kT_sb[hp:hp + 64,
                             c * S + tci * 128: c * S + (tci + 1) * 128]
                for sb in range(2):
                    nc.tensor.matmul(
                        st[:, sb * 512:(sb + 1) * 512], lhsT,
                        qT_sb[hp:hp + 64,
                              c * S + sb * 512: c * S + (sb + 1) * 512],
                        tile_position=(hp, 0))
                dst = exp_ab[half][:, tci * S:(tci + 1) * S]
                if half == 0:
                    nc.scalar.activation(dst, st[:], AF.Exp, scale=SCALE)
                else:
                    nc.vector.tensor_scalar(
                        dst.bitcast(I16), st[:], EXP_A, EXP_B,
                        op0=ALU.mult, op1=ALU.add)

        def qk_chain(c, w_sb, b_sb, o_sb):
            ps = mm_pool.tile([128, S], F32, tag="mm")
            for kc in range(NC_D):
                lhsT = w_sb[:, kc * D + c * 128: kc * D + (c + 1) * 128]
                for sb in range(2):
                    nc.tensor.matmul(
                        ps[:, sb * 512:(sb + 1) * 512], lhsT,
                        hsT_sb[:, kc * S + sb * 512: kc * S + (sb + 1) * 512],
                        start=(kc == 0), stop=(kc == NC_D - 1))
            nc.vector.tensor_scalar_add(o_sb[:, c * S:(c + 1) * S],
                                        ps[:], b_sb[:, c:c + 1])

        def v_chunk(sc):
            ps = mm_pool.tile([128, S], F32, tag="mm")
            for kc in range(NC_D):
                lhsT = hsT_sb[:, kc * S + sc * 128: kc * S + (sc + 1) * 128]
                nc.tensor.matmul(ps[:, 0:512], lhsT,
                                 wvT_sb[:, kc * VW: kc * VW + 512],
                                 start=(kc == 0), stop=(kc == NC_D - 1))
                nc.tensor.matmul(ps[:, 512:VW], lhsT,
                                 wvT_sb[:, kc * VW + 512: (kc + 1) * VW],
                                 start=(kc == 0), stop=(kc == NC_D - 1))
            vt = proj.tile([128, VW], F32, tag="vtmp", name=f"vt{sc}",
                           bufs=2)
            nc.vector.tensor_add(vt[:], ps[:, 0:VW], bvaug_sb[:])
            nc.vector.tensor_scalar_mul(v_sb[:, sc * VW:(sc + 1) * VW],
                                        vt[:], emask_sb[:, sc:sc + 1])

        # ---- QK proj chunk 0, then pair-0 scores with QK-chain spacers ----
        qk_chain(0, wqT_sb, bq_sb, qT_sb)
        qk_chain(0, wkT_sb, bk_sb, kT_sb)
        qk_spacers = []
        for c in range(1, NC_D):
            qk_spacers.append(lambda c=c: qk_chain(c, wqT_sb, bq_sb, qT_sb))
            qk_spacers.append(lambda c=c: qk_chain(c, wkT_sb, bk_sb, kT_sb))
        for tci in range(NC_S):
            scores_tci(0, tci, exps[0])
            n = 2 if tci < 2 else 1
            for _ in range(n):
                if qk_spacers:
                    qk_spacers.pop(0)()

        # ---- prompt K projection (transposed) ----
        for c in range(NC_D):
            ps = mm_pool.tile([128, S], F32, tag="mm")
            for kc in range(NC_D):
                nc.tensor.matmul(
                    ps[:, 0:AT],
                    wkT_sb[:, kc * D + c * 128: kc * D + (c + 1) * 128],
                    pT_sb[:, kc * AT:(kc + 1) * AT],
                    start=(kc == 0), stop=(kc == NC_D - 1))
            nc.vector.tensor_scalar_add(pkT_sb[:, c * AT:(c + 1) * AT],
                                        ps[:, 0:AT], bk_sb[:, c:c + 1])

        # ---- prompt V projection (natural, gate-scaled, duplicated) ----
        ps = mm_pool.tile([128, S], F32, tag="mm")
        for kc in range(NC_D):
            lhsT = pT_sb[:, kc * AT:(kc + 1) * AT]
            nc.tensor.matmul(ps[0:AT, 0:512], lhsT,
                             wvT_sb[:, kc * VW: kc * VW + 512],
                             start=(kc == 0), stop=(kc == NC_D - 1))
            nc.tensor.matmul(ps[0:AT, 512:VW], lhsT,
                             wvT_sb[:, kc * VW + 512: (kc + 1) * VW],
                             start=(kc == 0), stop=(kc == NC_D - 1))
        nc.vector.tensor_add(pvtmp_sb[:], ps[0:AT, 0:VW], bvaug_sb[0:AT, :])
        nc.vector.tensor_mul(pv_sb[0:AT, :], pvtmp_sb[:], gbc_sb[0:AT, :])
        nc.sync.dma_start(pv_sb[AT:128, :], pv_sb[0:AT, :])

        # ---- entire prefix branch, V chunks as PE spacers ----
        # per pair: prefix scores -> exp -> prefix ctx (ones column gives
        # the prefix denominator) -> bf16 evacuation + reciprocal chain to
        # DRAM (broadcast back during the attention phase).
        pe_ev = {}
        rdp = {}
        vq = list(range(NC_S))
        for c in range(PAIRS):
            if vq:
                v_chunk(vq.pop(0))
            psp = scp["p"].tile([128, S], F32, tag="sc", name=f"psp{c}")
            for half in range(2):
                hp = half * 64
                for sb in range(2):
                    nc.tensor.matmul(
                        psp[hp:hp + 64, sb * 512:(sb + 1) * 512],
                        pkT_sb[hp:hp + 64, c * AT:(c + 1) * AT],
                        qT_sb[hp:hp + 64,
                              c * S + sb * 512: c * S + (sb + 1) * 512],
                        tile_position=(hp, hp))
            pexp = pexp_pool.tile([128, S], BF16, tag="pexp",
                                  name=f"pexp{c}")
            nc.scalar.activation(pexp[:], psp[:], AF.Exp, scale=SCALE)
            if vq:
                v_chunk(vq.pop(0))
            dresh = proj.tile([128, 16], BF16, tag="drp", bufs=3,
                              name=f"drp{c}")
            for half in range(2):
                h = 2 * c + half
                hp = half * 64
                pps = scp["p"].tile([128, S], F32, tag="sc",
                                    name=f"pps{c}_{half}")
                for sb in range(2):
                    nc.tensor.matmul(
                        pps[0:65, sb * 512:(sb + 1) * 512],
                        pv_sb[hp:hp + 64, h * 65: h * 65 + 65],
                        pexp[hp:hp + 64, sb * 512:(sb + 1) * 512],
                        tile_position=(hp, 0))
                ev = pep_pool.tile([65, S], BF16, tag=f"pe{c}_{half}")
                with nc.allow_low_precision(
                        reason="prefix ctx to bf16: 0.4%% on the gated "
                               "prefix branch only"):
                    nc.scalar.copy(ev[:], pps[0:65, :])
                pe_ev[(c, half)] = ev
                nc.sync.dma_start(dresh[:, half * 8:(half + 1) * 8],
                                  ev[64:65, :])
            rrec = proj.tile([128, 16], BF16, tag="rrp", bufs=3,
                             name=f"rrp{c}")
            with nc.allow_low_precision(
                    reason="prefix denominator reciprocal in bf16"):
                nc.vector.reciprocal(rrec[:], dresh[:])
            rd = dscr_pool.tile([1, 2 * S], BF16, tag=f"rdp{c}", bufs=1,
                                name=f"rdp{c}")
            nc.sync.dma_start(rd[0:1, 0:S], rrec[:, 0:8])
            nc.sync.dma_start(rd[0:1, S:2 * S], rrec[:, 8:16])
            rdp[c] = rd
        while vq:
            v_chunk(vq.pop(0))

        sc0_cm.__exit__(None, None, None)
        proj_cm.__exit__(None, None, None)
        mm_cm.__exit__(None, None, None)

        # ---- attention-phase pools ----
        # banks 0-5: score rotation (3 x [128,1024]); banks 6-7: the two
        # ctx accumulators ([65,512] per head, s-halves sequential).
        scp["p"] = ctx.enter_context(
            tc.tile_pool(name="scp", bufs=3, space="PSUM"))
        ctx_pool = ctx.enter_context(
            tc.tile_pool(name="ctxp", bufs=1, space="PSUM"))
        norm_pool = ctx.enter_context(tc.tile_pool(name="normp", bufs=2))
        out_pool = ctx.enter_context(tc.tile_pool(name="outp", bufs=2))

        pend = {}

        def ctx_mm(p, half, k, sb):
            h = 2 * p + half
            lhsT = v_sb[:, k * VW + h * 65: k * VW + h * 65 + 65]
            nc.tensor.matmul(
                pend[p]["cps"][sb][half][0:65, :], lhsT,
                exps[p][half][:, k * S + sb * 512: k * S + (sb + 1) * 512],
                start=(k == 0), stop=(k == NC_S - 1))

        def ctx_alloc(p, sb):
            pend[p]["cps"][sb] = [
                ctx_pool.tile([65, 512], F32, tag="cA",
                              name=f"cps{p}_{sb}_0"),
                ctx_pool.tile([65, 512], F32, tag="cB",
                              name=f"cps{p}_{sb}_1")]

        def evac(p, sb):
            """Copy this s-half's accumulators (with denominator rows) to
            SBUF on ACT right after its last accumulation matmul — the
            banks free in ~1.5us, no DMA in the bank-recycle path. All
            normalization runs next block from the copies."""
            st = pend[p]
            if sb == 0:
                st["dresh"] = norm_pool.tile([128, 16], F32, tag="dresh",
                                             bufs=4, name=f"dr{p}")
                st["ce_ev"] = [[None, None], [None, None]]
            for half in range(2):
                ev = norm_pool.tile([65, 512], F32, tag="cev", bufs=8,
                                    name=f"cev{p}_{sb}_{half}")
                nc.scalar.copy(ev[:], st["cps"][sb][half][0:65, :])
                st["ce_ev"][sb][half] = ev
                nc.sync.dma_start(
                    st["dresh"][:, 8 * half + 4 * sb: 8 * half + 4 * sb + 4],
                    ev[64:65, :])

        def recips(q):
            """Reciprocal + DRAM broadcast of all four main denominators
            and the prefix pair; latency-tolerant (consumed 2+ chunks
            later)."""
            st = pend[q]
            st["rbc"] = []
            rrec = norm_pool.tile([128, 16], F32, tag="rrec", bufs=4,
                                  name=f"rr{q}")
            nc.vector.reciprocal(rrec[:], st["dresh"][:])
            for half in range(2):
                rd = dscr_pool.tile([1, S], F32, tag="rdm", bufs=4,
                                    name=f"rd{q}_{half}")
                nc.sync.dma_start(rd[0:1, 0:512],
                                  rrec[:, 8 * half: 8 * half + 4])
                nc.sync.dma_start(rd[0:1, 512:1024],
                                  rrec[:, 8 * half + 4: 8 * half + 8])
                rbc = norm_pool.tile([64, S], F32, tag="rbc", bufs=4,
                                     name=f"rbc{q}_{half}")
                r_src = bass.AP(rd[:].tensor, rd[:].offset,
                                [[0, 64], [1, S]])
                nc.sync.dma_start(rbc[:], r_src)
                st["rbc"].append(rbc)
            rbp = norm_pool.tile([64, 2 * S], BF16, tag="rbp", bufs=4,
                                 name=f"rbp{q}")
            rd_p = rdp[q]
            r_src = bass.AP(rd_p[:].tensor, rd_p[:].offset,
                            [[0, 64], [1, 2 * S]])
            nc.sync.dma_start(rbp[:], r_src)
            st["rbp"] = rbp
            st["ce_n"] = [None, None]

        def cemuls(q, half):
            """Normalize one head's ctx from the SBUF copies (DVE)."""
            st = pend[q]
            ce_n = out_pool.tile([64, S], F32, tag="ce", bufs=4,
                                 name=f"ce{q}_{half}")
            for sb in range(2):
                nc.vector.tensor_mul(
                    ce_n[:, sb * 512:(sb + 1) * 512],
                    st["ce_ev"][sb][half][0:64, :],
                    st["rbc"][half][:, sb * 512:(sb + 1) * 512])
            st["ce_n"][half] = ce_n

        def pe_ot(q, half, step):
            """Prefix normalize then combine + store (GpSimd)."""
            st = pend[q]
            h = 2 * q + half
            if step == 0:
                pe_n = out_pool.tile([64, S], F32, tag="pe", bufs=2,
                                     name=f"pen{h}")
                nc.gpsimd.tensor_mul(
                    pe_n[:], pe_ev[(q, half)][0:64, :],
                    st["rbp"][:, half * S:(half + 1) * S])
                st[f"pe_n{half}"] = pe_n
            else:
                ot = out_pool.tile([64, S], F32, tag="ot", bufs=2,
                                   name=f"ot{h}")
                nc.gpsimd.tensor_add(ot[:], st["ce_n"][half][:],
                                     st[f"pe_n{half}"][:])
                nc.sync.dma_start(outT[h * 64:(h + 1) * 64, :], ot[:])

        def attention_block(c):
            """Scores for pair c + ctx accumulation/evacuation for pair
            c-1 + normalize/combine/store for pair c-2."""
            p, q = c - 1, c - 2
            pend[p] = {"cps": [None, None]}
            for tci in range(NC_S):
                if tci == 0 and q >= 0:
                    recips(q)
                scores_tci(c, tci, exps[c])
                if tci == 1:
                    ctx_alloc(p, 0)
                if 1 <= tci <= 3:
                    for k in range(3 * (tci - 1), min(3 * tci, 8)):
                        ctx_mm(p, 0, k, 0)
                        ctx_mm(p, 1, k, 0)
                if tci == 2 and q >= 0:
                    cemuls(q, 0)
                elif tci == 3:
                    evac(p, 0)
                    if q >= 0:
                        cemuls(q, 1)
                elif tci == 4 and q >= 0:
                    pe_ot(q, 0, 0)
                elif tci == 5:
                    ctx_alloc(p, 1)
                    for k in range(4):
                        ctx_mm(p, 0, k, 1)
                        ctx_mm(p, 1, k, 1)
                    if q >= 0:
                        pe_ot(q, 0, 1)
                elif tci == 6:
                    for k in range(4, 8):
                        ctx_mm(p, 0, k, 1)
                        ctx_mm(p, 1, k, 1)
                    evac(p, 1)
                    if q >= 0:
                        pe_ot(q, 1, 0)
                elif tci == 7 and q >= 0:
                    pe_ot(q, 1, 1)

        for c in range(1, PAIRS):
            exps[c] = [exp_pool.tile([128, NC_S * S], BF16, tag="exp",
                                     name=f"exp_{c}_{i}") for i in range(2)]
            attention_block(c)

        # ---- tail: ctx(5) + remaining finishes ----
        p, q = PAIRS - 1, PAIRS - 2
        pend[p] = {"cps": [None, None]}
        recips(q)
        ctx_alloc(p, 0)
        for k in range(8):
            ctx_mm(p, 0, k, 0)
            ctx_mm(p, 1, k, 0)
            if k == 3:
                cemuls(q, 0)
            elif k == 5:
                cemuls(q, 1)
            elif k == 6:
                pe_ot(q, 0, 0)
            elif k == 7:
                pe_ot(q, 0, 1)
        evac(p, 0)
        pe_ot(q, 1, 0)
        ctx_alloc(p, 1)
        for k in range(8):
            ctx_mm(p, 0, k, 1)
            ctx_mm(p, 1, k, 1)
            if k == 3:
                pe_ot(q, 1, 1)
        evac(p, 1)
        recips(p)
        cemuls(p, 0)
        cemuls(p, 1)
        pe_ot(p, 0, 0)
        pe_ot(p, 0, 1)
        pe_ot(p, 1, 0)
        pe_ot(p, 1, 1)


def _prep_inputs(hidden_states, prompt_tokens, gating_factor, attention_mask,
                 Wq, bq, Wk, bk, Wv, bv):
    bf = ml_dtypes.bfloat16
    hs = np.asarray(hidden_states, np.float32)
    mask = np.asarray(attention_mask, np.float32).reshape(B, S)
    wqT = np.ascontiguousarray(np.asarray(Wq, np.float32).T).astype(bf)
    wkT = np.ascontiguousarray(np.asarray(Wk, np.float32).T).astype(bf)
    # augmented WvT: [din, 780], col 65h+j = Wv.T[:, 64h+j], col 65h+64 = 0
    wvT_f = np.asarray(Wv, np.float32).T  # [din, dout]
    wvT_aug = np.zeros((D, VW), np.float32)
    idx = np.arange(D)
    aug_cols = (idx // DH) * (DH + 1) + (idx % DH)
    wvT_aug[:, aug_cols] = wvT_f
    wvT_aug = wvT_aug.astype(bf)
    bq_c = np.asarray(bq, np.float32).reshape(D, 1)
    bk_c = np.asarray(bk, np.float32).reshape(D, 1)
    bv_aug = np.zeros(VW, np.float32)
    bv_aug[aug_cols] = np.asarray(bv, np.float32)
    bv_aug[DH::DH + 1] = 1.0
    bvaug_bc = np.ascontiguousarray(
        np.broadcast_to(bv_aug, (128, VW)), np.float32)
    pT = np.ascontiguousarray(
        np.asarray(prompt_tokens, np.float32)[0].T).astype(bf)
    gat_row = np.repeat(
        np.asarray(gating_factor, np.float32).reshape(H), DH + 1)
    gat = np.ascontiguousarray(
        np.broadcast_to(gat_row, (128, VW)), np.float32)

    shared = dict(wqT=wqT, wkT=wkT, wvT=wvT_aug, bq=bq_c, bk=bk_c,
                  bvaug=bvaug_bc, promptT=pT, gating=gat)
    in_maps = []
    for b in range(B):
        m = dict(shared)
        m["hsT"] = np.ascontiguousarray(hs[b].T).astype(bf)
        m["mask"] = np.ascontiguousarray(mask[b].reshape(S, 1))
        in_maps.append(m)
    return in_maps


def kernel(**inputs):
    global LAST_RESULTS
    if "nc" not in _CACHE:
        _CACHE["nc"] = _build_nc()
    nc = _CACHE["nc"]
    in_maps = _prep_inputs(**inputs)
    res = None
    for attempt in range(3):
        try:
            res = run_bass_kernel_spmd(nc, in_maps, list(range(B)))
            break
        except ModuleNotFoundError:
            import os

            os.environ["BASS_NEVER_TRACE"] = "1"
            if attempt == 2:
                raise
        except Exception:
            if attempt == 2:
                raise
    LAST_RESULTS = res
    out = np.empty((B, S, D), np.float32)
    for b in range(B):
        out[b] = res.results[b]["outT"].T
    return out
